# revision 1
# baseline (speedup 1.0000x reference)
"""DifferentialDropout Trainium2 kernel (8-core SPMD).

Reference semantics (see problem): per-row corrcoef factor, global-standardized
1000-bin per-row histograms -> entropies -> per-row keep prob -> mask+scale.

Sharding:
  Phase A (D-shard): each core takes a 2048-col slice of temp [1024, 16384]:
    partial row sums / global sum/sumsq/min/max (AllGather + local reduce),
    partial cov = xs @ xs.T via PE (K-sharded GEMM) -> AllReduce (overlapped
    with phase B histogram). Centering applied post-AR as rank-1 correction:
    cov = xxT - outer(rowsum, rowsum)/D.
  Phase B (B-shard): each core owns 128 rows: exact per-row 1000-bin histogram
    (radix 32x32: bf16 one-hot planes on DVE, combined per 128-element k-tile
    by PE matmuls accumulating [32q, 32l] counts in PSUM), entropies, factors,
    and the final mask/scale pass.
"""

import sys

sys.path.insert(0, "/opt/trn_rl_repo")

import numpy as np
import os

B = 1024
D = 16384
BINS = 1000
N_CORES = 8
DSL = D // N_CORES      # 2048
RSL = B // N_CORES      # 128
import os
C_ROWS = int(os.environ.get("K_CROWS", "4"))   # rows per histogram chunk
ABL = os.environ.get("K_ABL", "")              # ablation: skip phases in sim
DEBUG = os.environ.get("K_DEBUG", "0") == "1"
N_GPS = int(os.environ.get("K_NGPS", "14"))    # L-planes on gpsimd
N_ACT = int(os.environ.get("K_NACT", "8"))     # L-planes on scalar engine
EW_BUFS = int(os.environ.get("K_EWB", "10"))
F = C_ROWS * 128        # free elems/partition/chunk
N_CH = RSL // C_ROWS
N_CH_RUN = int(os.environ.get("K_NCH", "0")) or None
LN2 = 0.6931471805599453

_cache = {}


def _build():
    import concourse.mybir as mybir
    import concourse.tile as tile
    from concourse import bacc
    from concourse.masks import make_identity

    F32 = mybir.dt.float32
    BF16 = mybir.dt.bfloat16
    A = mybir.AluOpType
    AF = mybir.ActivationFunctionType
    AX = mybir.AxisListType.X

    nc = bacc.Bacc("TRN2", target_bir_lowering=False, debug=False,
                   num_devices=N_CORES)

    xst = nc.dram_tensor("xst", [DSL, B], F32, kind="ExternalInput")
    xr = nc.dram_tensor("xr", [RSL, D], F32, kind="ExternalInput")
    ur = nc.dram_tensor("ur", [RSL, D], F32, kind="ExternalInput")
    sel = nc.dram_tensor("sel", [128, 8], F32, kind="ExternalInput")
    out = nc.dram_tensor("out", [RSL, D], F32, kind="ExternalOutput")
    if DEBUG:
        dbg = nc.dram_tensor("dbg", [128, 16], F32, kind="ExternalOutput")
        dbg2 = nc.dram_tensor("dbg2", [128, 48], F32, kind="ExternalOutput")
        dbg3 = nc.dram_tensor("dbg3", [128, 12], F32, kind="ExternalOutput")

    xr_v = xr.ap().rearrange("r (p e) -> p r e", p=128)   # [128, 128, 128]

    with tile.TileContext(nc) as tc:
        with (
            tc.tile_pool(name="const", bufs=1) as constp,
            tc.tile_pool(name="persist", bufs=1) as persist,
            tc.tile_pool(name="dram", bufs=1, space="DRAM") as dram,
        ):
            id128 = constp.tile([128, 128], F32, name="id128")
            make_identity(nc, id128[:])
            ones32 = constp.tile([32, 1], F32, name="ones32")
            nc.vector.memset(ones32[:], 1.0)
            epsb = constp.tile([128, 1], F32, name="epsb")
            nc.vector.memset(epsb[:], 1e-30)
            one1 = constp.tile([128, 1], F32, name="one1")
            nc.vector.memset(one1[:], 1.0)
            negq = constp.tile([128, 32], F32, name="negq")
            for _q in range(32):
                nc.vector.memset(negq[:, _q:_q + 1], -float(_q))

            # persistent SBUF
            counts_sb = persist.tile([32, RSL * 32], F32, name="counts_sb")
            scal = persist.tile([128, 24], F32, name="scal")  # scalar consts
            ag_sb = persist.tile([128, 16], F32, name="ag_sb")
            agg_sb = persist.tile([128, 8, 16], F32, name="agg_sb")
            rowsum = persist.tile([128, 8], F32, name="rowsum")
            rsb = persist.tile([128, 1024], F32, name="rsb")
            rdb = persist.tile([128, 1024], F32, name="rdb")
            dcol = persist.tile([128, 8], F32, name="dcol")
            f1col = persist.tile([128, 8], F32, name="f1col")
            sel_sb = persist.tile([128, 8], F32, name="sel_sb")
            pvec = persist.tile([128, 4], F32, name="pvec")  # p, rkeep, keep, f1own
            bc_part = persist.tile([32, 32], F32, name="bc_part")

            nc.sync.dma_start(sel_sb[:], sel.ap())

            # DRAM bounces
            ag_in = dram.tile([128, 16], F32, name="ag_in")
            ag_out = dram.tile([1024, 16], F32, addr_space="Shared", name="ag_out")
            cov_in = dram.tile([1024, 1024], F32, name="cov_in")
            cov_out = dram.tile([128, 1024], F32, name="cov_out")
            agd_in = dram.tile([128, 33], F32, name="agd_in")
            agd_out = dram.tile([1024, 33], F32, addr_space="Shared", name="agd_out")

            # ---------------- Phase A: stats + GEMM --------------------------
            do_a = "A" not in ABL
            do_b = "B" not in ABL
            do_c = "C" not in ABL
            with (
                tc.tile_pool(name="pa_io", bufs=3) as pa_io,
                tc.tile_pool(name="pa_big", bufs=1) as pa_big,
                tc.tile_pool(name="pa_ps2", bufs=2, space="PSUM") as pa_ps2,
                tc.tile_pool(name="pa_ps", bufs=2, space="PSUM") as pa_ps,
                tc.tile_pool(name="pa_w", bufs=2) as pa_w,
            ):
                # stats over own rows (xr): rowsum, sumsq, min, max -> ag_sb[128,16]
                xst_sb = pa_big.tile([128, 16, 1024], F32, name="xst_sb")
                for k in range(16):
                    nc.sync.dma_start(xst_sb[:, k, :],
                                      xst.ap()[k * 128:(k + 1) * 128, :])
                nc.vector.memset(ag_sb[:, 8:12], 3.4e38)
                nc.vector.memset(ag_sb[:, 12:16], -3.4e38)
                SCH = 4096
                for sc_ in range(4):
                    xrs = pa_io.tile([128, SCH], F32, name="xrs")
                    nc.sync.dma_start(xrs[:], xr.ap()[:, sc_ * SCH:(sc_ + 1) * SCH])
                    rs_scr = pa_w.tile([128, SCH], F32, name="rs_scr", tag="scr", bufs=2)
                    nc.vector.tensor_scalar(rs_scr[:], xrs[:], 1.0, 0.0, A.mult, A.add,
                                            accum_out=ag_sb[:, sc_:sc_ + 1])
                    sq_scr = pa_w.tile([128, SCH], F32, name="sq_scr", tag="scr", bufs=2)
                    nc.scalar.activation(sq_scr[:], xrs[:], AF.Square,
                                         accum_out=ag_sb[:, 4 + sc_:5 + sc_])
                    nc.vector.reduce_sum(ag_sb[:, 8 + sc_:9 + sc_], xrs[:], axis=AX,
                                         op=A.min)
                    nc.vector.reduce_sum(ag_sb[:, 12 + sc_:13 + sc_], xrs[:], axis=AX,
                                         op=A.max)
                nc.sync.dma_start(ag_in[:], ag_sb[:])
                nc.gpsimd.collective_compute(
                    "AllGather", A.bypass,
                    replica_groups=[list(range(N_CORES))],
                    ins=[ag_in.opt()], outs=[ag_out.opt()])
                nc.sync.dma_start(
                    agg_sb[:], ag_out[:].rearrange("(r p) c -> p r c", p=128))

                # GEMM on host-transposed slice
                covp = pa_big.tile([128, 8, 1024], F32, name="covp")
                for m in range(8 if "G" not in ABL else 1):
                    for n2 in range(2):
                        pg = pa_ps2.tile([128, 512], F32, name="pg")
                        for k in range(16):
                            nc.tensor.matmul(
                                pg[:],
                                xst_sb[:, k, m * 128:(m + 1) * 128],
                                xst_sb[:, k, n2 * 512:(n2 + 1) * 512],
                                start=(k == 0), stop=(k == 15))
                        nc.scalar.copy(covp[:, m, n2 * 512:(n2 + 1) * 512], pg[:])
                nc.sync.dma_start(
                    cov_in[:].rearrange("(m p) j -> p m j", p=128), covp[:])
                nc.gpsimd.collective_compute(
                    "ReduceScatter", A.add,
                    replica_groups=[list(range(N_CORES))],
                    ins=[cov_in.opt()], outs=[cov_out.opt()])

                # -------- stats reduction + scalar constants --------
                nc.vector.reduce_sum(
                    rowsum[:], agg_sb[:, :, 0:4], axis=AX)
                nc.vector.reduce_sum(scal[:, 16:17], rowsum[:], axis=AX)
                nc.gpsimd.partition_all_reduce(scal[:, 0:1], scal[:, 16:17], 128,
                                               _reduce_add())
                nc.vector.reduce_sum(
                    scal[:, 17:18], agg_sb[:, :, 4:8], axis=mybir.AxisListType.XY)
                nc.gpsimd.partition_all_reduce(scal[:, 1:2], scal[:, 17:18], 128,
                                               _reduce_add())
                nc.vector.reduce_sum(
                    scal[:, 18:19], agg_sb[:, :, 8:12], axis=mybir.AxisListType.XY,
                    op=A.min)
                nc.vector.tensor_single_scalar(scal[:, 18:19], scal[:, 18:19], -1.0, A.mult)
                nc.gpsimd.partition_all_reduce(scal[:, 2:3], scal[:, 18:19], 128,
                                               _reduce_max())
                nc.vector.reduce_sum(
                    scal[:, 19:20], agg_sb[:, :, 12:16], axis=mybir.AxisListType.XY,
                    op=A.max)
                nc.gpsimd.partition_all_reduce(scal[:, 3:4], scal[:, 19:20], 128,
                                               _reduce_max())

                N_f = float(B) * float(D)
                # mu = gsum/N
                nc.vector.tensor_single_scalar(scal[:, 4:5], scal[:, 0:1], 1.0 / N_f, A.mult)
                # var = (gss - gsum*mu)/(N-1); sd = sqrt
                nc.vector.tensor_mul(scal[:, 20:21], scal[:, 0:1], scal[:, 4:5])
                nc.vector.tensor_sub(scal[:, 20:21], scal[:, 1:2], scal[:, 20:21])
                nc.vector.tensor_single_scalar(scal[:, 20:21], scal[:, 20:21],
                                               1.0 / (N_f - 1.0), A.mult)
                nc.scalar.activation(scal[:, 5:6], scal[:, 20:21], AF.Sqrt)
                nc.vector.reciprocal(scal[:, 6:7], scal[:, 5:6])
                # lo = (tmin - mu)*rsd ; tmin = -negmn
                nc.vector.tensor_single_scalar(scal[:, 21:22], scal[:, 2:3], -1.0, A.mult)
                nc.vector.tensor_sub(scal[:, 21:22], scal[:, 21:22], scal[:, 4:5])
                nc.vector.tensor_mul(scal[:, 7:8], scal[:, 21:22], scal[:, 6:7])
                # hi = (tmax - mu)*rsd ; width = (hi - lo)/BINS
                nc.vector.tensor_sub(scal[:, 22:23], scal[:, 3:4], scal[:, 4:5])
                nc.vector.tensor_mul(scal[:, 22:23], scal[:, 22:23], scal[:, 6:7])
                nc.vector.tensor_sub(scal[:, 22:23], scal[:, 22:23], scal[:, 7:8])
                nc.vector.tensor_single_scalar(scal[:, 8:9], scal[:, 22:23],
                                               1.0 / BINS, A.mult)
                nc.vector.reciprocal(scal[:, 9:10], scal[:, 8:9])
                # SC = rsd*rwidth ; BC = -(mu*rsd + lo)*rwidth
                nc.vector.tensor_mul(scal[:, 10:11], scal[:, 6:7], scal[:, 9:10])
                nc.vector.tensor_mul(scal[:, 23:24], scal[:, 4:5], scal[:, 6:7])
                nc.vector.tensor_add(scal[:, 23:24], scal[:, 23:24], scal[:, 7:8])
                nc.vector.tensor_mul(scal[:, 23:24], scal[:, 23:24], scal[:, 9:10])
                nc.vector.tensor_single_scalar(scal[:, 11:12], scal[:, 23:24], -1.0, A.mult)
                # entropy consts: rnw_l = 1/(width*D), rnw_b = 1/(width*N)
                nc.vector.tensor_single_scalar(scal[:, 16:17], scal[:, 8:9], float(D), A.mult)
                nc.vector.reciprocal(scal[:, 12:13], scal[:, 16:17])
                nc.vector.tensor_single_scalar(scal[:, 17:18], scal[:, 8:9], N_f, A.mult)
                nc.vector.reciprocal(scal[:, 13:14], scal[:, 17:18])
                nc.vector.tensor_single_scalar(scal[:, 14:15], scal[:, 12:13],
                                               -1.0 / LN2, A.mult)
                nc.vector.tensor_single_scalar(scal[:, 15:16], scal[:, 13:14],
                                               -1.0 / LN2, A.mult)
                # rsb = broadcast of flattened rowsum
                _bcast_cols(nc, pa_w, pa_ps, rowsum, rsb, id128)

            # ---------------- Phase B: histogram ----------------------------
            with (
                tc.tile_pool(name="hb_io", bufs=3) as hb_io,
                tc.tile_pool(name="hb_w", bufs=2) as hb_w,
                tc.tile_pool(name="hb_pl", bufs=2) as hb_pl,
                tc.tile_pool(name="hb_ps", bufs=8, space="PSUM") as hb_ps,
            ):
                for ch in range(min(N_CH, N_CH_RUN or N_CH) if do_b else 0):
                    r0 = ch * C_ROWS
                    xch = hb_io.tile([128, C_ROWS, 128], F32, name="xch")
                    nc.sync.dma_start(xch[:], xr_v[:, r0:r0 + C_ROWS, :])
                    xf = xch[:].rearrange("p a b -> p (a b)")

                    v = hb_w.tile([128, F], F32, name="v", tag="ew", bufs=EW_BUFS)
                    nc.scalar.activation(v[:], xf, AF.Identity,
                                         bias=scal[:, 11:12], scale=scal[:, 10:11])
                    i1 = hb_w.tile([128, F], mybir.dt.int32, name="i1", tag="ew", bufs=EW_BUFS)
                    nc.vector.tensor_copy(i1[:], v[:])
                    f1t = hb_w.tile([128, F], F32, name="f1t", tag="ew", bufs=EW_BUFS)
                    nc.vector.tensor_copy(f1t[:], i1[:])
                    g1 = hb_w.tile([128, F], F32, name="g1", tag="ew", bufs=EW_BUFS)
                    nc.vector.tensor_tensor(g1[:], f1t[:], v[:], A.is_gt)
                    idx = hb_w.tile([128, F], F32, name="idx", tag="ew", bufs=EW_BUFS)
                    nc.vector.tensor_sub(idx[:], f1t[:], g1[:])
                    i3 = hb_w.tile([128, F], mybir.dt.int32, name="i3", tag="ew", bufs=EW_BUFS)
                    nc.vector.tensor_scalar(i3[:], idx[:], 999.0, 0.0, A.min, A.max)
                    ih = hb_w.tile([128, F], mybir.dt.int32, name="ih", tag="ew", bufs=EW_BUFS)
                    nc.vector.tensor_single_scalar(ih[:], i3[:], 5, A.arith_shift_right)
                    il = hb_w.tile([128, F], mybir.dt.int32, name="il", tag="ew", bufs=EW_BUFS)
                    nc.vector.tensor_single_scalar(il[:], i3[:], 31, A.bitwise_and)
                    hi_b = hb_w.tile([128, F], BF16, name="hi_b")
                    nc.vector.tensor_copy(hi_b[:], ih[:])
                    lo_b = hb_w.tile([128, F], BF16, name="lo_b")
                    nc.vector.tensor_copy(lo_b[:], il[:])

                    Hpl = hb_pl.tile([128, 32 * F], BF16, name="Hpl")
                    Lpl = hb_pl.tile([128, 32 * F], BF16, name="Lpl")
                    for q in range(32):
                        nc.vector.tensor_single_scalar(
                            Hpl[:, q * F:(q + 1) * F], hi_b[:], float(q), A.is_equal)
                    for q in range(N_GPS):
                        nc.gpsimd.tensor_single_scalar(
                            Lpl[:, q * F:(q + 1) * F], lo_b[:], float(q), A.is_equal)
                    for q in range(N_GPS, N_GPS + N_ACT):
                        atmp = hb_w.tile([128, F], BF16, name="atmp")
                        nc.scalar.activation(atmp[:], lo_b[:], AF.Square,
                                             bias=negq[:, q:q + 1])
                        nc.scalar.activation(Lpl[:, q * F:(q + 1) * F], atmp[:],
                                             AF.Relu, bias=one1[:], scale=-1.0)
                    for q in range(N_GPS + N_ACT, 32):
                        nc.vector.tensor_single_scalar(
                            Lpl[:, q * F:(q + 1) * F], lo_b[:], float(q), A.is_equal)

                    Hv = Hpl[:].rearrange("p (q f) -> p f q", q=32)
                    Lv = Lpl[:].rearrange("p (q f) -> p f q", q=32)
                    for r in range(C_ROWS):
                        ps = hb_ps.tile([32, 32], F32, name="ps")
                        for k in range(128):
                            t = r * 128 + k
                            nc.tensor.matmul(ps[:], Hv[:, t, :], Lv[:, t, :],
                                             start=(k == 0), stop=(k == 127))
                        rr = r0 + r
                        nc.scalar.copy(counts_sb[:, rr * 32:(rr + 1) * 32], ps[:32, :])

                # batch-count partial (joins the merged AllGather in phase C)
                nc.vector.reduce_sum(
                    bc_part[:],
                    counts_sb[:].rearrange("p (r l) -> p l r", r=RSL), axis=AX)

            # ---------------- Phase C: factors + entropies + mask -----------
            with (
                tc.tile_pool(name="pc_w", bufs=2) as pc_w,
                tc.tile_pool(name="pc_big", bufs=1) as pc_big,
                tc.tile_pool(name="pc_ps", bufs=2, space="PSUM") as pc_ps,
                tc.tile_pool(name="pc_io", bufs=4) as pc_io,
            ):
                covc = pc_big.tile([128, 1024], F32, name="covc")
                nc.sync.dma_start(covc[:], cov_out[:])
                # rs_own / centering
                rs_own = pc_w.tile([128, 1], F32, name="rs_own", bufs=1)
                tsel = pc_w.tile([128, 8], F32, name="tsel", bufs=1)
                nc.vector.tensor_mul(tsel[:], rowsum[:], sel_sb[:])
                nc.vector.reduce_sum(rs_own[:], tsel[:], axis=AX)
                nc.vector.tensor_single_scalar(rs_own[:], rs_own[:], -1.0 / float(D), A.mult)
                nc.vector.scalar_tensor_tensor(covc[:], rsb[:], rs_own[:], covc[:],
                                               A.mult, A.add)
                # diag mask: dmask[p, t*128+q] = sel[p, t] * (q == p)
                dmask = pc_big.tile([128, 1024], F32, name="dmask")
                for t in range(8):
                    nc.vector.tensor_single_scalar(
                        dmask[:, t * 128:(t + 1) * 128], id128[:], sel_sb[:, t:t + 1],
                        A.mult)
                dtmp = pc_w.tile([128, 1024], F32, name="dtmp")
                nc.vector.tensor_mul(dtmp[:], covc[:], dmask[:])
                d_own = pc_w.tile([128, 1], F32, name="d_own", bufs=1)
                nc.vector.reduce_sum(d_own[:], dtmp[:], axis=AX)
                nc.scalar.activation(d_own[:], d_own[:], AF.Sqrt)
                agm = pc_w.tile([128, 33], F32, name="agm", bufs=1)
                nc.vector.memset(agm[:], 0.0)
                nc.scalar.copy(agm[:, 0:1], d_own[:])
                nc.scalar.copy(agm[0:32, 1:33], bc_part[:])
                nc.sync.dma_start(agd_in[:], agm[:])
                nc.gpsimd.collective_compute(
                    "AllGather", A.bypass,
                    replica_groups=[list(range(N_CORES))],
                    ins=[agd_in.opt()], outs=[agd_out.opt()])
                nc.sync.dma_start(
                    dcol[:].rearrange("p (t o) -> p t o", o=1),
                    agd_out[:].rearrange("(t p) c -> p t c", p=128)[:, :, 0:1])
                rdc = pc_w.tile([128, 8], F32, name="rdc", bufs=1)
                nc.vector.reciprocal(rdc[:], dcol[:])
                _bcast_cols(nc, pc_w, pc_ps, rdc, rdb, id128)
                rd_own = pc_w.tile([128, 1], F32, name="rd_own", bufs=1)
                nc.vector.reciprocal(rd_own[:], d_own[:])
                t1 = pc_w.tile([128, 1024], F32, name="t1")
                nc.vector.tensor_mul(t1[:], covc[:], rdb[:])
                nc.vector.tensor_single_scalar(t1[:], t1[:], rd_own[:], A.mult)
                t1c = pc_w.tile([128, 1024], F32, name="t1c")
                nc.scalar.activation(t1c[:], t1[:], AF.Abs, accum_out=pvec[:, 3:4])
                nc.vector.tensor_single_scalar(pvec[:, 3:4], pvec[:, 3:4],
                                               1.0 / float(B), A.mult)

                # local entropies
                lnch = pc_big.tile([32, RSL * 32], F32, name="lnch")
                nc.scalar.activation(lnch[:], counts_sb[:], AF.Ln,
                                     scale=scal[0:32, 12:13], bias=epsb[0:32, :])
                nc.vector.tensor_mul(lnch[:], lnch[:], counts_sb[:])
                erp = pc_w.tile([32, RSL], F32, name="erp", bufs=1)
                nc.vector.reduce_sum(
                    erp[:], lnch[:].rearrange("p (r l) -> p r l", r=RSL), axis=AX)
                psS = pc_ps.tile([1, RSL], F32, name="psS")
                nc.tensor.matmul(psS[:], ones32[:], erp[:], start=True, stop=True)
                srow = pc_w.tile([1, RSL], F32, name="srow", bufs=1)
                nc.scalar.copy(srow[:], psS[:])
                psT = pc_ps.tile([128, 1], F32, name="psT")
                nc.tensor.transpose(psT[:], srow[:], id128[:1, :1])
                hloc = pc_w.tile([128, 1], F32, name="hloc", bufs=1)
                nc.scalar.copy(hloc[:], psT[:])
                nc.vector.tensor_mul(hloc[:], hloc[:], scal[:, 14:15])

                # batch entropy: sum gathered bc blocks (core r rows r*128..+32)
                agb = pc_w.tile([32, 8, 32], F32, name="agb", bufs=1)
                nc.sync.dma_start(
                    agb[:], agd_out[:].rearrange("(r p) c -> p r c", p=128)[0:32, :, 1:33])
                bcs = pc_w.tile([32, 32], F32, name="bcs", bufs=1)
                nc.vector.reduce_sum(
                    bcs[:], agb[:].rearrange("p r c -> p c r"), axis=AX)
                lnb = pc_w.tile([32, 32], F32, name="lnb", bufs=1)
                nc.scalar.activation(lnb[:], bcs[:], AF.Ln,
                                     scale=scal[0:32, 13:14], bias=epsb[0:32, :])
                nc.vector.tensor_mul(lnb[:], lnb[:], bcs[:])
                sb1 = pc_w.tile([32, 1], F32, name="sb1", bufs=1)
                nc.vector.reduce_sum(sb1[:], lnb[:], axis=AX)
                nc.gpsimd.partition_all_reduce(sb1[:], sb1[:], 32, _reduce_add())
                hbat = pc_w.tile([128, 1], F32, name="hbat", bufs=1)
                nc.gpsimd.partition_broadcast(hbat[:], sb1[0:1, :])
                nc.vector.tensor_mul(hbat[:], hbat[:], scal[:, 15:16])

                # f2' = max(f2, 1/f2); keep = f1/f2'; p = 1-keep; rkeep = 1/keep
                tA = pc_w.tile([128, 1], F32, name="tA", bufs=1)
                tB = pc_w.tile([128, 1], F32, name="tB", bufs=1)
                nc.vector.reciprocal(tA[:], hbat[:])
                f2 = pc_w.tile([128, 1], F32, name="f2", bufs=1)
                nc.vector.tensor_mul(f2[:], hloc[:], tA[:])
                nc.vector.reciprocal(tB[:], f2[:])
                nc.vector.tensor_max(f2[:], f2[:], tB[:])
                nc.vector.reciprocal(tB[:], f2[:])
                nc.vector.tensor_mul(pvec[:, 2:3], pvec[:, 3:4], tB[:])
                nc.vector.tensor_scalar(pvec[:, 0:1], pvec[:, 2:3], -1.0, 1.0,
                                        A.mult, A.add)
                nc.vector.reciprocal(pvec[:, 1:2], pvec[:, 2:3])

                if DEBUG:
                    dbg_sb = pc_w.tile([128, 16], F32, name="dbg_sb", bufs=1)
                    nc.scalar.copy(dbg_sb[:, 0:4], pvec[:])
                    nc.scalar.copy(dbg_sb[:, 4:5], hloc[:])
                    nc.scalar.copy(dbg_sb[:, 5:6], hbat[:])
                    nc.scalar.copy(dbg_sb[:, 6:14], scal[:, 4:12])
                    nc.scalar.copy(dbg_sb[:, 14:16], scal[:, 12:14])
                    nc.sync.dma_start(dbg.ap(), dbg_sb[:])
                    dbg2_sb = pc_w.tile([128, 48], F32, name="dbg2_sb", bufs=1)
                    nc.scalar.copy(dbg2_sb[:, 0:8], dcol[:])
                    nc.scalar.copy(dbg2_sb[:, 8:16], rdc[:])
                    nc.scalar.copy(dbg2_sb[:, 16:24], rsb[:, 0:8])
                    nc.scalar.copy(dbg2_sb[:, 24:32], rdb[:, 0:8])
                    nc.scalar.copy(dbg2_sb[:, 32:40], covc[:, 0:8])
                    nc.scalar.copy(dbg2_sb[:, 40:48], rowsum[:])
                    nc.sync.dma_start(dbg2.ap(), dbg2_sb[:])
                    dbg3_sb = pc_w.tile([128, 12], F32, name="dbg3_sb", bufs=1)
                    nc.scalar.copy(dbg3_sb[:, 0:4], t1[:, 0:4])
                    nc.scalar.copy(dbg3_sb[:, 4:8], covc[:, 0:4])
                    nc.scalar.copy(dbg3_sb[:, 8:12], rdc[:, 0:4])
                    nc.sync.dma_start(dbg3.ap(), dbg3_sb[:])

                # mask + scale
                CH = 2048
                for c in range((D // CH) if do_c else 0):
                    xm = pc_io.tile([128, CH], F32, name="xm")
                    um = pc_io.tile([128, CH], F32, name="um")
                    nc.sync.dma_start(xm[:], xr.ap()[:, c * CH:(c + 1) * CH])
                    nc.sync.dma_start(um[:], ur.ap()[:, c * CH:(c + 1) * CH])
                    nc.vector.tensor_single_scalar(um[:], um[:], pvec[:, 0:1], A.is_gt)
                    oc = pc_io.tile([128, CH], F32, name="oc")
                    nc.vector.scalar_tensor_tensor(oc[:], um[:], pvec[:, 1:2], xm[:],
                                                   A.mult, A.mult)
                    nc.sync.dma_start(out.ap()[:, c * CH:(c + 1) * CH], oc[:])

    nc.compile()
    return nc


def _reduce_add():
    from concourse import bass_isa
    return bass_isa.ReduceOp.add


def _reduce_max():
    from concourse import bass_isa
    return bass_isa.ReduceOp.max


def _bcast_cols(nc, sbuf_pool, psum_pool, vec8, dst, id128):
    """dst[p, t*128+q] = vec8[q, t]  (flatten [128,8] col-major, bcast to all
    partitions)."""
    import concourse.mybir as mybir
    F32 = mybir.dt.float32
    pt = psum_pool.tile([8, 128], F32, name="bc_pt")
    nc.tensor.transpose(pt[:8, :], vec8[:], id128[:])
    tr = sbuf_pool.tile([8, 128], F32, name="bc_tr", bufs=1)
    nc.scalar.copy(tr[:], pt[:8, :])
    flat = sbuf_pool.tile([1, 8 * 128], F32, name="bc_flat", bufs=1)
    for t in range(8):
        nc.sync.dma_start(flat[:, t * 128:(t + 1) * 128], tr[t:t + 1, :])
    nc.gpsimd.partition_broadcast(dst[:], flat[:])


def kernel(x, u):
    if "nc" not in _cache:
        _cache["nc"] = _build()
    nc = _cache["nc"]
    from concourse.bass_utils import run_bass_kernel_spmd

    x = np.asarray(x, dtype=np.float32)
    u = np.asarray(u, dtype=np.float32)
    orig_shape = x.shape
    xf = np.ascontiguousarray(x.reshape(B, D))
    uf = np.ascontiguousarray(u.reshape(B, D))
    in_maps = []
    for c in range(N_CORES):
        selv = np.zeros((128, 8), np.float32)
        selv[:, c] = 1.0
        in_maps.append({
            "xst": np.ascontiguousarray(xf[:, c * DSL:(c + 1) * DSL].T),
            "xr": np.ascontiguousarray(xf[c * RSL:(c + 1) * RSL, :]),
            "ur": np.ascontiguousarray(uf[c * RSL:(c + 1) * RSL, :]),
            "sel": selv,
        })
    res = run_bass_kernel_spmd(nc, in_maps, core_ids=list(range(N_CORES)))
    _cache["last_results"] = res
    outf = np.concatenate([res.results[c]["out"] for c in range(N_CORES)], axis=0)
    return outf.reshape(orig_shape)



# revision 5
# speedup vs baseline: 1.2848x; 1.2848x over previous
"""DifferentialDropout Trainium2 kernel (8-core SPMD), v2.

Reference semantics: per-row corrcoef factor f1, global-standardized 1000-bin
per-row histograms -> entropies -> per-row keep prob -> mask+scale.

v2 changes vs baseline (771.8 us):
  - cov GEMM in float32r (1 cycle/row vs fp32's 4) with explicit rounding
    copies; rowsums via PE matmul on the same f32r tiles.
  - phase A stats computed from the xst (D-shard) tiles while they stream in;
    ONE combined AllGather carries rowsum partials + min/max/sumsq.
  - histogram prep: fused floor chain (6 DVE ops/chunk) writing a combined
    z = [hi | lo] tensor; 32 double-width is_equal planes (vs 64 single).
  - plane generation split DVE/Act/Pool via tunables; Act uses Square+Relu.
  - d (corr diag) AllGather issued mid-phase-B (hidden); per-chunk local
    entropy partials (tail only does the tiny batch-entropy exchange).
  - mask-phase x/u loads issued before the batch-count exchange (prefetch).
"""

import os
import sys

sys.path.insert(0, "/opt/trn_rl_repo")

import numpy as np

B = 1024
D = 16384
BINS = 1000
N_CORES = 8
DSL = D // N_CORES      # 2048
RSL = B // N_CORES      # 128
C_ROWS = 4
F = C_ROWS * 128        # 512 elems/partition/chunk
N_CH = RSL // C_ROWS    # 32 chunks
LN2 = 0.6931471805599453

# plane split: 32 double-planes per chunk across engines
PL_DVE = int(os.environ.get("K_PLDVE", "23"))
PL_ACT = int(os.environ.get("K_PLACT", "3"))
PL_GPS = 32 - PL_DVE - PL_ACT
EW_BUFS = int(os.environ.get("K_EWB", "8"))
DEBUG = os.environ.get("K_DEBUG", "0") == "1"

_cache = {}


def _build():
    import concourse.mybir as mybir
    import concourse.tile as tile
    from concourse import bacc
    from concourse.masks import make_identity

    F32 = mybir.dt.float32
    F32R = mybir.dt.float32r
    BF16 = mybir.dt.bfloat16
    I32 = mybir.dt.int32
    A = mybir.AluOpType
    AF = mybir.ActivationFunctionType
    AX = mybir.AxisListType.X
    AXY = mybir.AxisListType.XY

    nc = bacc.Bacc("TRN2", target_bir_lowering=False, debug=False,
                   num_devices=N_CORES)

    xst = nc.dram_tensor("xst", [DSL, B], F32, kind="ExternalInput")
    xr = nc.dram_tensor("xr", [RSL, D], F32, kind="ExternalInput")
    ur = nc.dram_tensor("ur", [RSL, D], F32, kind="ExternalInput")
    sel = nc.dram_tensor("sel", [128, 8], F32, kind="ExternalInput")
    out = nc.dram_tensor("out", [RSL, D], F32, kind="ExternalOutput")
    if DEBUG:
        dbg = nc.dram_tensor("dbg", [128, 16], F32, kind="ExternalOutput")

    xr_v = xr.ap().rearrange("r (p e) -> p r e", p=128)   # [128, 128, 128]

    with tile.TileContext(nc) as tc:
        with (
            tc.tile_pool(name="const", bufs=1) as constp,
            tc.tile_pool(name="persist", bufs=1) as persist,
            tc.tile_pool(name="dram", bufs=1, space="DRAM") as dram,
        ):
            id128 = constp.tile([128, 128], F32, name="id128")
            make_identity(nc, id128[:])
            ones32 = constp.tile([32, 1], F32, name="ones32")
            nc.vector.memset(ones32[:], 1.0)
            ones1f = constp.tile([128, 1], F32, name="ones1f")
            nc.vector.memset(ones1f[:], 1.0)
            ones1r = constp.tile([128, 1], F32R, name="ones1r")
            nc.vector.tensor_copy(ones1r[:], ones1f[:])
            epsb = constp.tile([128, 1], F32, name="epsb")
            nc.vector.memset(epsb[:], 1e-30)
            one1 = constp.tile([128, 1], F32, name="one1")
            nc.vector.memset(one1[:], 1.0)
            negq = constp.tile([128, 32], F32, name="negq")
            for _q in range(32):
                nc.vector.memset(negq[:, _q:_q + 1], -float(_q))

            # persistent SBUF
            counts_sb = persist.tile([32, RSL * 32], F32, name="counts_sb")
            erp_all = persist.tile([32, RSL], F32, name="erp_all")
            scal = persist.tile([128, 24], F32, name="scal")
            rsb = persist.tile([128, 1024], F32, name="rsb")
            rdb = persist.tile([128, 1024], F32, name="rdb")
            dmask = persist.tile([128, 1024], F32, name="dmask")
            covc = persist.tile([128, 1024], F32, name="covc")
            sel_sb = persist.tile([128, 8], F32, name="sel_sb")
            rs_own = persist.tile([128, 1], F32, name="rs_own")
            d_own = persist.tile([128, 1], F32, name="d_own")
            rd_own = persist.tile([128, 1], F32, name="rd_own")
            f1own = persist.tile([128, 1], F32, name="f1own")
            pvec = persist.tile([128, 4], F32, name="pvec")

            nc.sync.dma_start(sel_sb[:], sel.ap())

            # DRAM bounces
            ag_in = dram.tile([1, 1032], F32, name="ag_in")
            ag_out = dram.tile([8, 1032], F32, addr_space="Shared", name="ag_out")
            cov_in = dram.tile([1024, 1024], F32, name="cov_in")
            cov_out = dram.tile([128, 1024], F32, name="cov_out")
            d_in = dram.tile([128, 1], F32, name="d_in")
            d_out = dram.tile([1024, 1], F32, addr_space="Shared", name="d_out")
            bc_in = dram.tile([32, 32], F32, name="bc_in")
            bc_out = dram.tile([256, 32], F32, addr_space="Shared", name="bc_out")

            # ---------------- Phase A: stats + GEMM (f32r) -------------------
            with (
                tc.tile_pool(name="pa_io", bufs=3) as pa_io,
                tc.tile_pool(name="pa_big", bufs=1) as pa_big,
                tc.tile_pool(name="pa_w", bufs=2) as pa_w,
                tc.tile_pool(name="pa_ps", bufs=2, space="PSUM") as pa_ps,
                tc.tile_pool(name="pa_ps2", bufs=2, space="PSUM") as pa_ps2,
                tc.tile_pool(name="pa_rs", bufs=1, space="PSUM") as pa_rs,
            ):
                xst_r = pa_big.tile([128, 16, 1024], F32R, name="xst_r")
                mn16 = pa_big.tile([128, 16], F32, name="mn16")
                mx16 = pa_big.tile([128, 16], F32, name="mx16")
                ssq16 = pa_big.tile([128, 16], F32, name="ssq16")
                rs_ps = pa_rs.tile([1, 2, 512], F32, name="rs_ps")
                for k in range(16):
                    xk = pa_io.tile([128, 1024], F32, name="xk")
                    nc.sync.dma_start(xk[:], xst.ap()[k * 128:(k + 1) * 128, :])
                    # f32r rounding copy (split across Pool/DVE)
                    if k % 2 == 0:
                        nc.gpsimd.tensor_copy(xst_r[:, k, :], xk[:])
                    else:
                        nc.vector.tensor_copy(xst_r[:, k, :], xk[:])
                    nc.vector.reduce_sum(mn16[:, k:k + 1], xk[:], axis=AX, op=A.min)
                    nc.vector.reduce_sum(mx16[:, k:k + 1], xk[:], axis=AX, op=A.max)
                    sq = pa_w.tile([128, 1024], F32, name="sq", tag="sq", bufs=2)
                    nc.scalar.activation(sq[:], xk[:], AF.Square,
                                         accum_out=ssq16[:, k:k + 1])
                    # rowsum partial via PE (accumulate over k)
                    nc.tensor.matmul(rs_ps[:, 0, :], ones1r[:], xst_r[:, k, 0:512],
                                     start=(k == 0), stop=(k == 15))
                    nc.tensor.matmul(rs_ps[:, 1, :], ones1r[:], xst_r[:, k, 512:1024],
                                     start=(k == 0), stop=(k == 15))

                # local reductions -> per-partition partials
                pmn = pa_w.tile([128, 1], F32, name="pmn", bufs=1)
                nc.vector.reduce_sum(pmn[:], mn16[:], axis=AX, op=A.min)
                nc.vector.tensor_single_scalar(pmn[:], pmn[:], -1.0, A.mult)
                pmx = pa_w.tile([128, 1], F32, name="pmx", bufs=1)
                nc.vector.reduce_sum(pmx[:], mx16[:], axis=AX, op=A.max)
                pss = pa_w.tile([128, 1], F32, name="pss", bufs=1)
                nc.vector.reduce_sum(pss[:], ssq16[:], axis=AX)
                # cross-partition (128) reduce
                nc.gpsimd.partition_all_reduce(pmn[:], pmn[:], 128, _rop("max"))
                nc.gpsimd.partition_all_reduce(pmx[:], pmx[:], 128, _rop("max"))
                nc.gpsimd.partition_all_reduce(pss[:], pss[:], 128, _rop("add"))
                rs_sb = pa_w.tile([1, 1024], F32, name="rs_sb", bufs=1)
                nc.scalar.copy(rs_sb[:, 0:512], rs_ps[:, 0, :])
                nc.scalar.copy(rs_sb[:, 512:1024], rs_ps[:, 1, :])
                nc.sync.dma_start(ag_in[:, 0:1024], rs_sb[:])
                nc.sync.dma_start(ag_in[:, 1024:1025], pmn[0:1, :])
                nc.sync.dma_start(ag_in[:, 1025:1026], pmx[0:1, :])
                nc.sync.dma_start(ag_in[:, 1026:1027], pss[0:1, :])
                nc.gpsimd.collective_compute(
                    "AllGather", A.bypass,
                    replica_groups=[list(range(N_CORES))],
                    ins=[ag_in.opt()], outs=[ag_out.opt()])

                # GEMM: covp = xs @ xs.T (K-sharded partial), f32r
                covp = pa_big.tile([128, 8, 1024], F32, name="covp")
                for m in range(8):
                    for n2 in range(2):
                        pg = pa_ps2.tile([128, 512], F32, name="pg")
                        for k in range(16):
                            nc.tensor.matmul(
                                pg[:],
                                xst_r[:, k, m * 128:(m + 1) * 128],
                                xst_r[:, k, n2 * 512:(n2 + 1) * 512],
                                start=(k == 0), stop=(k == 15))
                        nc.scalar.copy(covp[:, m, n2 * 512:(n2 + 1) * 512], pg[:])
                nc.sync.dma_start(
                    cov_in[:].rearrange("(m p) j -> p m j", p=128), covp[:])
                nc.gpsimd.collective_compute(
                    "ReduceScatter", A.add,
                    replica_groups=[list(range(N_CORES))],
                    ins=[cov_in.opt()], outs=[cov_out.opt()])

                # -------- process gathered stats --------
                agsb = pa_w.tile([8, 1032], F32, name="agsb", bufs=1)
                nc.sync.dma_start(agsb[:], ag_out[:])
                rsfull = pa_w.tile([8, 1024], F32, name="rsfull", bufs=1)
                nc.gpsimd.partition_all_reduce(rsfull[:], agsb[:, 0:1024], 8,
                                               _rop("add"))
                g3 = pa_w.tile([8, 3], F32, name="g3", bufs=1)
                nc.gpsimd.partition_all_reduce(g3[:, 0:1], agsb[:, 1024:1025], 8,
                                               _rop("max"))
                nc.gpsimd.partition_all_reduce(g3[:, 1:2], agsb[:, 1025:1026], 8,
                                               _rop("max"))
                nc.gpsimd.partition_all_reduce(g3[:, 2:3], agsb[:, 1026:1027], 8,
                                               _rop("add"))
                nc.gpsimd.partition_broadcast(rsb[:], rsfull[0:1, :])
                nc.gpsimd.partition_broadcast(scal[:, 2:3], g3[0:1, 0:1])  # -min
                nc.gpsimd.partition_broadcast(scal[:, 3:4], g3[0:1, 1:2])  # max
                nc.gpsimd.partition_broadcast(scal[:, 1:2], g3[0:1, 2:3])  # ssq
                nc.vector.reduce_sum(scal[:, 0:1], rsb[:], axis=AX)        # gsum

                # -------- scalar constants (same math as baseline) --------
                N_f = float(B) * float(D)
                nc.vector.tensor_single_scalar(scal[:, 4:5], scal[:, 0:1], 1.0 / N_f, A.mult)
                nc.vector.tensor_mul(scal[:, 20:21], scal[:, 0:1], scal[:, 4:5])
                nc.vector.tensor_sub(scal[:, 20:21], scal[:, 1:2], scal[:, 20:21])
                nc.vector.tensor_single_scalar(scal[:, 20:21], scal[:, 20:21],
                                               1.0 / (N_f - 1.0), A.mult)
                nc.scalar.activation(scal[:, 5:6], scal[:, 20:21], AF.Sqrt)
                nc.vector.reciprocal(scal[:, 6:7], scal[:, 5:6])
                nc.vector.tensor_single_scalar(scal[:, 21:22], scal[:, 2:3], -1.0, A.mult)
                nc.vector.tensor_sub(scal[:, 21:22], scal[:, 21:22], scal[:, 4:5])
                nc.vector.tensor_mul(scal[:, 7:8], scal[:, 21:22], scal[:, 6:7])
                nc.vector.tensor_sub(scal[:, 22:23], scal[:, 3:4], scal[:, 4:5])
                nc.vector.tensor_mul(scal[:, 22:23], scal[:, 22:23], scal[:, 6:7])
                nc.vector.tensor_sub(scal[:, 22:23], scal[:, 22:23], scal[:, 7:8])
                nc.vector.tensor_single_scalar(scal[:, 8:9], scal[:, 22:23],
                                               1.0 / BINS, A.mult)
                nc.vector.reciprocal(scal[:, 9:10], scal[:, 8:9])
                nc.vector.tensor_mul(scal[:, 10:11], scal[:, 6:7], scal[:, 9:10])
                nc.vector.tensor_mul(scal[:, 23:24], scal[:, 4:5], scal[:, 6:7])
                nc.vector.tensor_add(scal[:, 23:24], scal[:, 23:24], scal[:, 7:8])
                nc.vector.tensor_mul(scal[:, 23:24], scal[:, 23:24], scal[:, 9:10])
                nc.vector.tensor_single_scalar(scal[:, 11:12], scal[:, 23:24], -1.0, A.mult)
                nc.vector.tensor_single_scalar(scal[:, 16:17], scal[:, 8:9], float(D), A.mult)
                nc.vector.reciprocal(scal[:, 12:13], scal[:, 16:17])
                nc.vector.tensor_single_scalar(scal[:, 17:18], scal[:, 8:9], N_f, A.mult)
                nc.vector.reciprocal(scal[:, 13:14], scal[:, 17:18])
                nc.vector.tensor_single_scalar(scal[:, 14:15], scal[:, 12:13],
                                               -1.0 / LN2, A.mult)
                nc.vector.tensor_single_scalar(scal[:, 15:16], scal[:, 13:14],
                                               -1.0 / LN2, A.mult)

                # dmask[p, t*128+q] = sel[p, t] * (q == p); rs_own
                for t in range(8):
                    nc.vector.tensor_single_scalar(
                        dmask[:, t * 128:(t + 1) * 128], id128[:],
                        sel_sb[:, t:t + 1], A.mult)
                rtmp = pa_w.tile([128, 1024], F32, name="rtmp", bufs=1)
                nc.vector.tensor_mul(rtmp[:], rsb[:], dmask[:])
                nc.vector.reduce_sum(rs_own[:], rtmp[:], axis=AX)
                nc.vector.tensor_single_scalar(rs_own[:], rs_own[:],
                                               -1.0 / float(D), A.mult)

            # ---------------- Phase B: histogram ----------------------------
            with (
                tc.tile_pool(name="hb_io", bufs=3) as hb_io,
                tc.tile_pool(name="hb_w", bufs=2) as hb_w,
                tc.tile_pool(name="hb_z", bufs=2) as hb_z,
                tc.tile_pool(name="hb_pl", bufs=2) as hb_pl,
                tc.tile_pool(name="hb_ps", bufs=6, space="PSUM") as hb_ps,
                tc.tile_pool(name="hb_ps2", bufs=1, space="PSUM") as hb_ps2,
            ):
                for ch in range(N_CH):
                    r0 = ch * C_ROWS
                    xch = hb_io.tile([128, C_ROWS, 128], F32, name="xch")
                    nc.sync.dma_start(xch[:], xr_v[:, r0:r0 + C_ROWS, :])
                    xf = xch[:].rearrange("p a b -> p (a b)")

                    # v = relu(SC*x + BC); fused floor chain
                    v = hb_w.tile([128, F], F32, name="v", tag="ew", bufs=EW_BUFS)
                    nc.scalar.activation(v[:], xf, AF.Relu,
                                         bias=scal[:, 11:12], scale=scal[:, 10:11])
                    i1 = hb_w.tile([128, F], I32, name="i1", tag="ew", bufs=EW_BUFS)
                    nc.vector.tensor_single_scalar(i1[:], v[:], 999.49, A.min)
                    g1 = hb_w.tile([128, F], F32, name="g1", tag="ew", bufs=EW_BUFS)
                    nc.vector.tensor_tensor(g1[:], i1[:], v[:], A.is_gt)
                    idxf = hb_w.tile([128, F], F32, name="idxf", tag="ew", bufs=EW_BUFS)
                    nc.vector.tensor_tensor(idxf[:], i1[:], g1[:], A.subtract)
                    ihi = hb_w.tile([128, F], I32, name="ihi", tag="ew", bufs=EW_BUFS)
                    nc.vector.tensor_scalar(ihi[:], idxf[:], 1.0 / 32.0, -0.484375,
                                            A.mult, A.add)
                    z = hb_z.tile([128, 2 * F], BF16, name="z")
                    nc.vector.tensor_copy(z[:, 0:F], ihi[:])
                    nc.vector.scalar_tensor_tensor(z[:, F:2 * F], ihi[:], -32.0,
                                                   idxf[:], A.mult, A.add)

                    # 32 double planes: P[:, j, 0:F] = (zh==j), P[:, j, F:2F] = (zl==j)
                    P = hb_pl.tile([128, 32, 2 * F], BF16, name="P")
                    for j in range(PL_DVE):
                        nc.vector.tensor_single_scalar(
                            P[:, j, :], z[:], float(j), A.is_equal)
                    for j in range(PL_DVE, PL_DVE + PL_GPS):
                        nc.gpsimd.tensor_single_scalar(
                            P[:, j, :], z[:], float(j), A.is_equal)
                    for j in range(PL_DVE + PL_GPS, 32):
                        atmp = hb_w.tile([128, 2 * F], BF16, name="atmp",
                                         tag="at", bufs=2)
                        nc.scalar.activation(atmp[:], z[:], AF.Square,
                                             bias=negq[:, j:j + 1])
                        nc.scalar.activation(P[:, j, :], atmp[:], AF.Relu,
                                             bias=one1[:], scale=-1.0)

                    for r in range(C_ROWS):
                        ps = hb_ps.tile([32, 32], F32, name="ps")
                        for k in range(128):
                            t = r * 128 + k
                            nc.tensor.matmul(ps[:], P[:, :, t], P[:, :, F + t],
                                             start=(k == 0), stop=(k == 127))
                        rr = r0 + r
                        nc.scalar.copy(counts_sb[:, rr * 32:(rr + 1) * 32],
                                       ps[:32, :])

                    # per-chunk local-entropy partial:
                    # erp_all[:, r] = sum_l c * ln(c*rnw_l + eps)
                    lnc = hb_w.tile([32, C_ROWS * 32], F32, name="lnc",
                                    tag="ln", bufs=2)
                    nc.scalar.activation(lnc[:], counts_sb[:, r0 * 32:(r0 + C_ROWS) * 32],
                                         AF.Ln, scale=scal[0:32, 12:13],
                                         bias=epsb[0:32, :])
                    nc.vector.tensor_mul(lnc[:], lnc[:],
                                         counts_sb[:, r0 * 32:(r0 + C_ROWS) * 32])
                    nc.vector.reduce_sum(
                        erp_all[:, r0:r0 + C_ROWS],
                        lnc[:].rearrange("p (r l) -> p r l", r=C_ROWS), axis=AX)

                    # mid-phase: cov post-processing + d exchange (hidden)
                    if ch == 10:
                        nc.sync.dma_start(covc[:], cov_out[:])
                        nc.vector.scalar_tensor_tensor(covc[:], rsb[:], rs_own[:],
                                                       covc[:], A.mult, A.add)
                        dtmp = hb_w.tile([128, 1024], F32, name="dtmp",
                                         tag="dt", bufs=2)
                        nc.vector.tensor_mul(dtmp[:], covc[:], dmask[:])
                        nc.vector.reduce_sum(d_own[:], dtmp[:], axis=AX)
                        nc.scalar.activation(d_own[:], d_own[:], AF.Sqrt)
                        nc.vector.reciprocal(rd_own[:], d_own[:])
                        nc.sync.dma_start(d_in[:], d_own[:])
                        nc.gpsimd.collective_compute(
                            "AllGather", A.bypass,
                            replica_groups=[list(range(N_CORES))],
                            ins=[d_in.opt()], outs=[d_out.opt()])
                    if ch == 14:
                        dcol = hb_w.tile([128, 8], F32, name="dcol", bufs=1)
                        nc.sync.dma_start(
                            dcol[:].rearrange("p (t o) -> p t o", o=1),
                            d_out[:].rearrange("(t p) c -> p t c", p=128))
                        rdc = hb_w.tile([128, 8], F32, name="rdc", bufs=1)
                        nc.vector.reciprocal(rdc[:], dcol[:])
                        _bcast_cols(nc, hb_w, hb_ps2, rdc, rdb, id128)
                    if ch == 17:
                        t1 = hb_w.tile([128, 1024], F32, name="t1", tag="dt", bufs=2)
                        nc.vector.tensor_mul(t1[:], covc[:], rdb[:])
                        nc.vector.tensor_single_scalar(t1[:], t1[:], rd_own[:],
                                                       A.mult)
                        t1c = hb_w.tile([128, 1024], F32, name="t1c", tag="dt", bufs=2)
                        nc.scalar.activation(t1c[:], t1[:], AF.Abs,
                                             accum_out=f1own[:])
                        nc.vector.tensor_single_scalar(f1own[:], f1own[:],
                                                       1.0 / float(B), A.mult)

            # ---------------- Phase C: entropies + mask ----------------------
            with (
                tc.tile_pool(name="pc_w", bufs=2) as pc_w,
                tc.tile_pool(name="pc_ps", bufs=2, space="PSUM") as pc_ps,
                tc.tile_pool(name="pc_io", bufs=6) as pc_io,
            ):
                # prefetch mask-phase loads (independent of pvec)
                xm_t = []
                um_t = []
                CH = 2048
                for c in range(D // CH):
                    xm = pc_io.tile([128, CH], F32, name="xm", tag="xm", bufs=3)
                    um = pc_io.tile([128, CH], F32, name="um", tag="um", bufs=3)
                    nc.sync.dma_start(xm[:], xr.ap()[:, c * CH:(c + 1) * CH])
                    nc.sync.dma_start(um[:], ur.ap()[:, c * CH:(c + 1) * CH])
                    xm_t.append(xm)
                    um_t.append(um)

                # batch-count partial + exchange
                bc_part = pc_w.tile([32, 32], F32, name="bc_part", bufs=1)
                nc.vector.reduce_sum(
                    bc_part[:],
                    counts_sb[:].rearrange("p (r l) -> p l r", r=RSL), axis=AX)
                nc.sync.dma_start(bc_in[:], bc_part[:])
                nc.gpsimd.collective_compute(
                    "AllGather", A.bypass,
                    replica_groups=[list(range(N_CORES))],
                    ins=[bc_in.opt()], outs=[bc_out.opt()])
                agb = pc_w.tile([32, 8, 32], F32, name="agb", bufs=1)
                nc.sync.dma_start(
                    agb[:], bc_out[:].rearrange("(r p) c -> p r c", p=32))
                bcs = pc_w.tile([32, 32], F32, name="bcs", bufs=1)
                nc.vector.reduce_sum(
                    bcs[:], agb[:].rearrange("p r c -> p c r"), axis=AX)
                lnb = pc_w.tile([32, 32], F32, name="lnb", bufs=1)
                nc.scalar.activation(lnb[:], bcs[:], AF.Ln,
                                     scale=scal[0:32, 13:14], bias=epsb[0:32, :])
                nc.vector.tensor_mul(lnb[:], lnb[:], bcs[:])
                sb1 = pc_w.tile([32, 1], F32, name="sb1", bufs=1)
                nc.vector.reduce_sum(sb1[:], lnb[:], axis=AX)
                nc.gpsimd.partition_all_reduce(sb1[:], sb1[:], 32, _rop("add"))
                hbat = pc_w.tile([128, 1], F32, name="hbat", bufs=1)
                nc.gpsimd.partition_broadcast(hbat[:], sb1[0:1, :])
                nc.vector.tensor_mul(hbat[:], hbat[:], scal[:, 15:16])

                # local entropies from accumulated partials
                psS = pc_ps.tile([1, RSL], F32, name="psS")
                nc.tensor.matmul(psS[:], ones32[:], erp_all[:], start=True,
                                 stop=True)
                srow = pc_w.tile([1, RSL], F32, name="srow", bufs=1)
                nc.scalar.copy(srow[:], psS[:])
                psT = pc_ps.tile([128, 1], F32, name="psT")
                nc.tensor.transpose(psT[:], srow[:], id128[:1, :1])
                hloc = pc_w.tile([128, 1], F32, name="hloc", bufs=1)
                nc.scalar.copy(hloc[:], psT[:])
                nc.vector.tensor_mul(hloc[:], hloc[:], scal[:, 14:15])

                # f2' = max(f2, 1/f2); keep = f1/f2'; p = 1-keep; rkeep = 1/keep
                tA = pc_w.tile([128, 1], F32, name="tA", bufs=1)
                tB = pc_w.tile([128, 1], F32, name="tB", bufs=1)
                nc.vector.reciprocal(tA[:], hbat[:])
                f2 = pc_w.tile([128, 1], F32, name="f2", bufs=1)
                nc.vector.tensor_mul(f2[:], hloc[:], tA[:])
                nc.vector.reciprocal(tB[:], f2[:])
                nc.vector.tensor_max(f2[:], f2[:], tB[:])
                nc.vector.reciprocal(tB[:], f2[:])
                nc.vector.tensor_mul(pvec[:, 2:3], f1own[:], tB[:])
                nc.vector.tensor_scalar(pvec[:, 0:1], pvec[:, 2:3], -1.0, 1.0,
                                        A.mult, A.add)
                nc.vector.reciprocal(pvec[:, 1:2], pvec[:, 2:3])

                if DEBUG:
                    dbg_sb = pc_w.tile([128, 16], F32, name="dbg_sb", bufs=1)
                    nc.scalar.copy(dbg_sb[:, 0:4], pvec[:])
                    nc.scalar.copy(dbg_sb[:, 4:5], hloc[:])
                    nc.scalar.copy(dbg_sb[:, 5:6], hbat[:])
                    nc.scalar.copy(dbg_sb[:, 6:14], scal[:, 4:12])
                    nc.scalar.copy(dbg_sb[:, 14:15], f1own[:])
                    nc.scalar.copy(dbg_sb[:, 15:16], d_own[:])
                    nc.sync.dma_start(dbg.ap(), dbg_sb[:])

                # mask + scale
                for c in range(D // CH):
                    xm, um = xm_t[c], um_t[c]
                    nc.vector.tensor_single_scalar(um[:], um[:], pvec[:, 0:1],
                                                   A.is_gt)
                    oc = pc_io.tile([128, CH], F32, name="oc", tag="oc", bufs=3)
                    nc.vector.scalar_tensor_tensor(oc[:], um[:], pvec[:, 1:2],
                                                   xm[:], A.mult, A.mult)
                    nc.sync.dma_start(out.ap()[:, c * CH:(c + 1) * CH], oc[:])

    nc.compile()
    return nc


def _rop(name):
    from concourse import bass_isa
    return getattr(bass_isa.ReduceOp, name)


def _bcast_cols(nc, sbuf_pool, psum_pool, vec8, dst, id128):
    """dst[p, t*128+q] = vec8[q, t]  (flatten [128,8] col-major, bcast to all
    partitions)."""
    import concourse.mybir as mybir
    F32 = mybir.dt.float32
    pt = psum_pool.tile([8, 128], F32, name="bc_pt")
    nc.tensor.transpose(pt[:8, :], vec8[:], id128[:])
    tr = sbuf_pool.tile([8, 128], F32, name="bc_tr", bufs=1)
    nc.scalar.copy(tr[:], pt[:8, :])
    flat = sbuf_pool.tile([1, 8 * 128], F32, name="bc_flat", bufs=1)
    for t in range(8):
        nc.sync.dma_start(flat[:, t * 128:(t + 1) * 128], tr[t:t + 1, :])
    nc.gpsimd.partition_broadcast(dst[:], flat[:])


def kernel(x, u):
    if "nc" not in _cache:
        _cache["nc"] = _build()
    nc = _cache["nc"]
    from concourse.bass_utils import run_bass_kernel_spmd

    x = np.asarray(x, dtype=np.float32)
    u = np.asarray(u, dtype=np.float32)
    orig_shape = x.shape
    xf = np.ascontiguousarray(x.reshape(B, D))
    uf = np.ascontiguousarray(u.reshape(B, D))
    in_maps = []
    for c in range(N_CORES):
        selv = np.zeros((128, 8), np.float32)
        selv[:, c] = 1.0
        in_maps.append({
            "xst": np.ascontiguousarray(xf[:, c * DSL:(c + 1) * DSL].T),
            "xr": np.ascontiguousarray(xf[c * RSL:(c + 1) * RSL, :]),
            "ur": np.ascontiguousarray(uf[c * RSL:(c + 1) * RSL, :]),
            "sel": selv,
        })
    res = run_bass_kernel_spmd(nc, in_maps, core_ids=list(range(N_CORES)))
    _cache["last_results"] = res
    outf = np.concatenate([res.results[c]["out"] for c in range(N_CORES)], axis=0)
    return outf.reshape(orig_shape)


# revision 9
# speedup vs baseline: 1.3621x; 1.0601x over previous
"""DifferentialDropout Trainium2 kernel (8-core SPMD), v3.

Reference semantics: per-row corrcoef factor f1, global-standardized 1000-bin
per-row histograms -> entropies -> per-row keep prob -> mask+scale.

Key design (vs 771.8 us baseline):
  - cov GEMM in float32r (1 cycle/row vs fp32's 4) with explicit rounding
    copies; rowsums via PE matmul on the same f32r tiles.
  - phase A stats computed from the xst (D-shard) tiles while they stream;
    ONE combined AllGather carries rowsum partials + min/max/sumsq, issued
    before the GEMM so phase B starts ~50 us.
  - histogram prep: fused floor chain (7 DVE ops/chunk) writing
    z2 = [hi | lo | hi-16 | lo-16]; ONE is_equal over z2 yields TWO planes
    (j, j+16) stored pair-interleaved (entropy is permutation-invariant
    over bins, so the scramble never needs undoing).
  - plane pairs split DVE/Act/Pool; Act uses Square+Relu pairs.
  - chunks 0-3 are Act-free (v on DVE, pairs redistributed, entropies
    deferred) so the GEMM's PSUM->SBUF staging copies can run on Act
    without head-blocking phase B; cov ReduceScatter issued at chunk 5,
    d AllGather at chunk 13 - all hidden under phase B.
  - mask-phase x/u loads prefetched before the batch-count exchange.
"""

import os
import sys

sys.path.insert(0, "/opt/trn_rl_repo")

import numpy as np

B = 1024
D = 16384
BINS = 1000
N_CORES = 8
DSL = D // N_CORES      # 2048
RSL = B // N_CORES      # 128
C_ROWS = 4
F = C_ROWS * 128        # 512 elems/partition/chunk
N_CH = RSL // C_ROWS    # 32 chunks
LN2 = 0.6931471805599453

# plane split: 16 pair-planes per chunk across engines
PR_DVE = int(os.environ.get("K_PRDVE", "11"))
PR_ACT = int(os.environ.get("K_PRACT", "2"))
PR_GPS = 16 - PR_DVE - PR_ACT
ACT_FREE_CH = int(os.environ.get("K_AFCH", "4"))   # Act-free leading chunks
EW_BUFS = int(os.environ.get("K_EWB", "6"))
DEBUG = os.environ.get("K_DEBUG", "0") == "1"

_cache = {}


def _build():
    import concourse.mybir as mybir
    import concourse.tile as tile
    from concourse import bacc
    from concourse.masks import make_identity

    F32 = mybir.dt.float32
    F32R = mybir.dt.float32r
    BF16 = mybir.dt.bfloat16
    I32 = mybir.dt.int32
    A = mybir.AluOpType
    AF = mybir.ActivationFunctionType
    AX = mybir.AxisListType.X

    nc = bacc.Bacc("TRN2", target_bir_lowering=False, debug=False,
                   num_devices=N_CORES)

    xst = nc.dram_tensor("xst", [DSL, B], F32, kind="ExternalInput")
    xr = nc.dram_tensor("xr", [RSL, D], F32, kind="ExternalInput")
    ur = nc.dram_tensor("ur", [RSL, D], F32, kind="ExternalInput")
    sel = nc.dram_tensor("sel", [128, 8], F32, kind="ExternalInput")
    out = nc.dram_tensor("out", [RSL, D], F32, kind="ExternalOutput")
    if DEBUG:
        dbg = nc.dram_tensor("dbg", [128, 16], F32, kind="ExternalOutput")

    xr_v = xr.ap().rearrange("r (p e) -> p r e", p=128)   # [128, 128, 128]

    with tile.TileContext(nc) as tc:
        with (
            tc.tile_pool(name="const", bufs=1) as constp,
            tc.tile_pool(name="persist", bufs=1) as persist,
            tc.tile_pool(name="dram", bufs=1, space="DRAM") as dram,
        ):
            id128 = constp.tile([128, 128], F32, name="id128")
            make_identity(nc, id128[:])
            ones32 = constp.tile([32, 1], F32, name="ones32")
            nc.vector.memset(ones32[:], 1.0)
            ones1f = constp.tile([128, 1], F32, name="ones1f")
            nc.vector.memset(ones1f[:], 1.0)
            ones1r = constp.tile([128, 1], F32R, name="ones1r")
            nc.vector.tensor_copy(ones1r[:], ones1f[:])
            epsb = constp.tile([128, 1], F32, name="epsb")
            nc.vector.memset(epsb[:], 1e-30)
            one1 = constp.tile([128, 1], F32, name="one1")
            nc.vector.memset(one1[:], 1.0)
            negq = constp.tile([128, 16], F32, name="negq")
            for _q in range(16):
                nc.vector.memset(negq[:, _q:_q + 1], -float(_q))

            # persistent SBUF
            counts_sb = persist.tile([32, RSL * 32], F32, name="counts_sb")
            erp_all = persist.tile([32, RSL], F32, name="erp_all")
            scal = persist.tile([128, 24], F32, name="scal")
            rsb = persist.tile([128, 1024], F32, name="rsb")
            rdb = persist.tile([128, 1024], F32, name="rdb")
            dmask = persist.tile([128, 1024], F32, name="dmask")
            covc = persist.tile([128, 1024], F32, name="covc")
            sel_sb = persist.tile([128, 8], F32, name="sel_sb")
            rs_own = persist.tile([128, 1], F32, name="rs_own")
            d_own = persist.tile([128, 1], F32, name="d_own")
            rd_own = persist.tile([128, 1], F32, name="rd_own")
            f1own = persist.tile([128, 1], F32, name="f1own")
            pvec = persist.tile([128, 4], F32, name="pvec")

            nc.sync.dma_start(sel_sb[:], sel.ap())

            # DRAM bounces
            ag_in = dram.tile([1, 1032], F32, name="ag_in")
            ag_out = dram.tile([8, 1032], F32, addr_space="Shared", name="ag_out")
            cov_in = dram.tile([1024, 1024], F32, name="cov_in")
            cov_out = dram.tile([128, 1024], F32, name="cov_out")
            d_in = dram.tile([128, 1], F32, name="d_in")
            d_out = dram.tile([1024, 1], F32, addr_space="Shared", name="d_out")
            bc_in = dram.tile([32, 32], F32, name="bc_in")
            bc_out = dram.tile([256, 32], F32, addr_space="Shared", name="bc_out")

            # ---------------- Phase A: stats + GEMM (f32r) -------------------
            with (
                tc.tile_pool(name="pa_io", bufs=3) as pa_io,
                tc.tile_pool(name="pa_big", bufs=1) as pa_big,
                tc.tile_pool(name="pa_w", bufs=2) as pa_w,
                tc.tile_pool(name="pa_ps2", bufs=2, space="PSUM") as pa_ps2,
                tc.tile_pool(name="pa_rs", bufs=1, space="PSUM") as pa_rs,
            ):
                xst_r = pa_big.tile([128, 16, 1024], F32R, name="xst_r")
                mn16 = pa_big.tile([128, 16], F32, name="mn16")
                mx16 = pa_big.tile([128, 16], F32, name="mx16")
                ssq16 = pa_big.tile([128, 16], F32, name="ssq16")
                rs_ps = pa_rs.tile([1, 2, 512], F32, name="rs_ps")
                for k in range(16):
                    xk = pa_io.tile([128, 1024], F32, name="xk")
                    nc.sync.dma_start(xk[:], xst.ap()[k * 128:(k + 1) * 128, :])
                    # f32r rounding copy on Pool; min+max reduces on DVE
                    nc.gpsimd.tensor_copy(xst_r[:, k, :], xk[:])
                    nc.vector.reduce_sum(mn16[:, k:k + 1], xk[:], axis=AX, op=A.min)
                    nc.vector.reduce_sum(mx16[:, k:k + 1], xk[:], axis=AX, op=A.max)
                    sq = pa_w.tile([128, 1024], F32, name="sq", tag="sq", bufs=2)
                    nc.scalar.activation(sq[:], xk[:], AF.Square,
                                         accum_out=ssq16[:, k:k + 1])
                    # rowsum partial via PE (accumulate over k)
                    nc.tensor.matmul(rs_ps[:, 0, :], ones1r[:], xst_r[:, k, 0:512],
                                     start=(k == 0), stop=(k == 15))
                    nc.tensor.matmul(rs_ps[:, 1, :], ones1r[:], xst_r[:, k, 512:1024],
                                     start=(k == 0), stop=(k == 15))

                # local reductions -> per-partition partials
                pmn = pa_w.tile([128, 1], F32, name="pmn", bufs=1)
                nc.vector.reduce_sum(pmn[:], mn16[:], axis=AX, op=A.min)
                nc.vector.tensor_single_scalar(pmn[:], pmn[:], -1.0, A.mult)
                pmx = pa_w.tile([128, 1], F32, name="pmx", bufs=1)
                nc.vector.reduce_sum(pmx[:], mx16[:], axis=AX, op=A.max)
                pss = pa_w.tile([128, 1], F32, name="pss", bufs=1)
                nc.vector.reduce_sum(pss[:], ssq16[:], axis=AX)
                # cross-partition (128) reduce
                nc.gpsimd.partition_all_reduce(pmn[:], pmn[:], 128, _rop("max"))
                nc.gpsimd.partition_all_reduce(pmx[:], pmx[:], 128, _rop("max"))
                nc.gpsimd.partition_all_reduce(pss[:], pss[:], 128, _rop("add"))
                rs_sb = pa_w.tile([1, 1024], F32, name="rs_sb", bufs=1)
                nc.scalar.copy(rs_sb[:, 0:512], rs_ps[:, 0, :])
                nc.scalar.copy(rs_sb[:, 512:1024], rs_ps[:, 1, :])
                nc.sync.dma_start(ag_in[:, 0:1024], rs_sb[:])
                nc.sync.dma_start(ag_in[:, 1024:1025], pmn[0:1, :])
                nc.sync.dma_start(ag_in[:, 1025:1026], pmx[0:1, :])
                nc.sync.dma_start(ag_in[:, 1026:1027], pss[0:1, :])
                nc.gpsimd.collective_compute(
                    "AllGather", A.bypass,
                    replica_groups=[list(range(N_CORES))],
                    ins=[ag_in.opt()], outs=[ag_out.opt()])

                # -------- process gathered stats --------
                agsb = pa_w.tile([8, 1032], F32, name="agsb", bufs=1)
                nc.sync.dma_start(agsb[:], ag_out[:])
                rsfull = pa_w.tile([8, 1024], F32, name="rsfull", bufs=1)
                nc.gpsimd.partition_all_reduce(rsfull[:], agsb[:, 0:1024], 8,
                                               _rop("add"))
                g3 = pa_w.tile([8, 3], F32, name="g3", bufs=1)
                nc.gpsimd.partition_all_reduce(g3[:, 0:1], agsb[:, 1024:1025], 8,
                                               _rop("max"))
                nc.gpsimd.partition_all_reduce(g3[:, 1:2], agsb[:, 1025:1026], 8,
                                               _rop("max"))
                nc.gpsimd.partition_all_reduce(g3[:, 2:3], agsb[:, 1026:1027], 8,
                                               _rop("add"))
                nc.gpsimd.partition_broadcast(rsb[:], rsfull[0:1, :])
                nc.gpsimd.partition_broadcast(scal[:, 2:3], g3[0:1, 0:1])  # -min
                nc.gpsimd.partition_broadcast(scal[:, 3:4], g3[0:1, 1:2])  # max
                nc.gpsimd.partition_broadcast(scal[:, 1:2], g3[0:1, 2:3])  # ssq
                nc.vector.reduce_sum(scal[:, 0:1], rsb[:], axis=AX)        # gsum

                # -------- scalar constants (same math as baseline) --------
                N_f = float(B) * float(D)
                nc.vector.tensor_single_scalar(scal[:, 4:5], scal[:, 0:1], 1.0 / N_f, A.mult)
                nc.vector.tensor_mul(scal[:, 20:21], scal[:, 0:1], scal[:, 4:5])
                nc.vector.tensor_sub(scal[:, 20:21], scal[:, 1:2], scal[:, 20:21])
                nc.vector.tensor_single_scalar(scal[:, 20:21], scal[:, 20:21],
                                               1.0 / (N_f - 1.0), A.mult)
                nc.scalar.activation(scal[:, 5:6], scal[:, 20:21], AF.Sqrt)
                nc.vector.reciprocal(scal[:, 6:7], scal[:, 5:6])
                nc.vector.tensor_single_scalar(scal[:, 21:22], scal[:, 2:3], -1.0, A.mult)
                nc.vector.tensor_sub(scal[:, 21:22], scal[:, 21:22], scal[:, 4:5])
                nc.vector.tensor_mul(scal[:, 7:8], scal[:, 21:22], scal[:, 6:7])
                nc.vector.tensor_sub(scal[:, 22:23], scal[:, 3:4], scal[:, 4:5])
                nc.vector.tensor_mul(scal[:, 22:23], scal[:, 22:23], scal[:, 6:7])
                nc.vector.tensor_sub(scal[:, 22:23], scal[:, 22:23], scal[:, 7:8])
                nc.vector.tensor_single_scalar(scal[:, 8:9], scal[:, 22:23],
                                               1.0 / BINS, A.mult)
                nc.vector.reciprocal(scal[:, 9:10], scal[:, 8:9])
                nc.vector.tensor_mul(scal[:, 10:11], scal[:, 6:7], scal[:, 9:10])
                nc.vector.tensor_mul(scal[:, 23:24], scal[:, 4:5], scal[:, 6:7])
                nc.vector.tensor_add(scal[:, 23:24], scal[:, 23:24], scal[:, 7:8])
                nc.vector.tensor_mul(scal[:, 23:24], scal[:, 23:24], scal[:, 9:10])
                nc.vector.tensor_single_scalar(scal[:, 11:12], scal[:, 23:24], -1.0, A.mult)
                nc.vector.tensor_single_scalar(scal[:, 16:17], scal[:, 8:9], float(D), A.mult)
                nc.vector.reciprocal(scal[:, 12:13], scal[:, 16:17])
                nc.vector.tensor_single_scalar(scal[:, 17:18], scal[:, 8:9], N_f, A.mult)
                nc.vector.reciprocal(scal[:, 13:14], scal[:, 17:18])
                nc.vector.tensor_single_scalar(scal[:, 14:15], scal[:, 12:13],
                                               -1.0 / LN2, A.mult)
                nc.vector.tensor_single_scalar(scal[:, 15:16], scal[:, 13:14],
                                               -1.0 / LN2, A.mult)

                # dmask[p, t*128+q] = sel[p, t] * (q == p); rs_own
                for t in range(8):
                    nc.vector.tensor_single_scalar(
                        dmask[:, t * 128:(t + 1) * 128], id128[:],
                        sel_sb[:, t:t + 1], A.mult)
                rtmp = pa_w.tile([128, 1024], F32, name="rtmp", bufs=1)
                nc.vector.tensor_mul(rtmp[:], rsb[:], dmask[:])
                nc.vector.reduce_sum(rs_own[:], rtmp[:], axis=AX)
                nc.vector.tensor_single_scalar(rs_own[:], rs_own[:],
                                               -1.0 / float(D), A.mult)

                # GEMM: covp = xs @ xs.T (K-sharded partial), f32r.
                # Staging copies run on Act (phase B chunks 0..3 are Act-free
                # so these never head-block histogram work).
                covp = pa_big.tile([128, 8, 1024], F32, name="covp")
                for m in range(8):
                    for n2 in range(2):
                        pg = pa_ps2.tile([128, 512], F32, name="pg")
                        for k in range(16):
                            nc.tensor.matmul(
                                pg[:],
                                xst_r[:, k, m * 128:(m + 1) * 128],
                                xst_r[:, k, n2 * 512:(n2 + 1) * 512],
                                start=(k == 0), stop=(k == 15))
                        nc.scalar.copy(covp[:, m, n2 * 512:(n2 + 1) * 512], pg[:])
                nc.sync.dma_start(
                    cov_in[:].rearrange("(m p) j -> p m j", p=128), covp[:])

            # ---------------- Phase B: histogram ----------------------------
            with (
                tc.tile_pool(name="hb_io", bufs=3) as hb_io,
                tc.tile_pool(name="hb_w", bufs=2) as hb_w,
                tc.tile_pool(name="hb_z", bufs=2) as hb_z,
                tc.tile_pool(name="hb_pl", bufs=2) as hb_pl,
                tc.tile_pool(name="hb_ps", bufs=6, space="PSUM") as hb_ps,
                tc.tile_pool(name="hb_ps2", bufs=1, space="PSUM") as hb_ps2,
            ):
                pend_ent = []

                def _entropy_partial(er0):
                    lnc = hb_w.tile([32, C_ROWS * 32], F32, name="lnc",
                                    tag="ln", bufs=2)
                    nc.scalar.activation(
                        lnc[:], counts_sb[:, er0 * 32:(er0 + C_ROWS) * 32],
                        AF.Ln, scale=scal[0:32, 12:13], bias=epsb[0:32, :])
                    nc.vector.tensor_mul(
                        lnc[:], lnc[:], counts_sb[:, er0 * 32:(er0 + C_ROWS) * 32])
                    nc.vector.reduce_sum(
                        erp_all[:, er0:er0 + C_ROWS],
                        lnc[:].rearrange("p (r l) -> p r l", r=C_ROWS), axis=AX)

                for ch in range(N_CH):
                    act_free = ch < ACT_FREE_CH
                    r0 = ch * C_ROWS
                    xch = hb_io.tile([128, C_ROWS, 128], F32, name="xch")
                    nc.sync.dma_start(xch[:], xr_v[:, r0:r0 + C_ROWS, :])
                    xf = xch[:].rearrange("p a b -> p (a b)")

                    # v = SC*x + BC (Act: +Relu; DVE fallback lets the rare
                    # v<0 element drop from the histogram, which is harmless)
                    v = hb_w.tile([128, F], F32, name="v", tag="ew", bufs=EW_BUFS)
                    if act_free:
                        nc.vector.tensor_scalar(v[:], xf, scal[:, 10:11],
                                                scal[:, 11:12], A.mult, A.add)
                    else:
                        nc.scalar.activation(v[:], xf, AF.Relu,
                                             bias=scal[:, 11:12],
                                             scale=scal[:, 10:11])
                    i1 = hb_w.tile([128, F], I32, name="i1", tag="ew", bufs=EW_BUFS)
                    nc.vector.tensor_single_scalar(i1[:], v[:], 999.49, A.min)
                    g1 = hb_w.tile([128, F], F32, name="g1", tag="ew", bufs=EW_BUFS)
                    nc.vector.tensor_tensor(g1[:], i1[:], v[:], A.is_gt)
                    idxf = hb_w.tile([128, F], F32, name="idxf", tag="ew", bufs=EW_BUFS)
                    nc.vector.tensor_tensor(idxf[:], i1[:], g1[:], A.subtract)
                    ihi = hb_w.tile([128, F], I32, name="ihi", tag="ew", bufs=EW_BUFS)
                    nc.vector.tensor_scalar(ihi[:], idxf[:], 1.0 / 32.0, -0.484375,
                                            A.mult, A.add)
                    z2 = hb_z.tile([128, 4 * F], BF16, name="z2")
                    nc.vector.tensor_copy(z2[:, 0:F], ihi[:])
                    nc.vector.scalar_tensor_tensor(z2[:, F:2 * F], ihi[:], -32.0,
                                                   idxf[:], A.mult, A.add)
                    nc.vector.tensor_single_scalar(z2[:, 2 * F:4 * F],
                                                   z2[:, 0:2 * F], 16.0,
                                                   A.subtract)

                    # pair planes: one is_equal over z2=[z | z-16] yields planes
                    # (j, j+16) stored interleaved at slots (2j, 2j+1).  All
                    # downstream uses of counts are permutation-invariant.
                    P = hb_pl.tile([128, 32, 2 * F], BF16, name="P")
                    Pv = P[:].rearrange("p q f -> p (q f)")
                    n_dve = PR_DVE + (1 if act_free else 0)
                    n_gps = PR_GPS + (1 if act_free else 0)
                    for j in range(n_dve):
                        nc.vector.tensor_single_scalar(
                            Pv[:, j * 4 * F:(j + 1) * 4 * F], z2[:], float(j),
                            A.is_equal)
                    for j in range(n_dve, n_dve + n_gps):
                        nc.gpsimd.tensor_single_scalar(
                            Pv[:, j * 4 * F:(j + 1) * 4 * F], z2[:], float(j),
                            A.is_equal)
                    for j in range(n_dve + n_gps, 16):
                        atmp = hb_w.tile([128, 4 * F], BF16, name="atmp",
                                         tag="at", bufs=1)
                        nc.scalar.activation(atmp[:], z2[:], AF.Square,
                                             bias=negq[:, j:j + 1])
                        nc.scalar.activation(Pv[:, j * 4 * F:(j + 1) * 4 * F],
                                             atmp[:], AF.Relu,
                                             bias=one1[:], scale=-1.0)

                    for r in range(C_ROWS):
                        ps = hb_ps.tile([32, 32], F32, name="ps")
                        for k in range(128):
                            t = r * 128 + k
                            nc.tensor.matmul(ps[:], P[:, :, t], P[:, :, F + t],
                                             start=(k == 0), stop=(k == 127))
                        rr = r0 + r
                        if act_free:
                            nc.vector.tensor_copy(
                                counts_sb[:, rr * 32:(rr + 1) * 32], ps[:32, :])
                        else:
                            nc.scalar.copy(
                                counts_sb[:, rr * 32:(rr + 1) * 32], ps[:32, :])

                    # per-chunk local-entropy partials (deferred while Act-free)
                    if act_free:
                        pend_ent.append(r0)
                    else:
                        for pr0 in pend_ent:
                            _entropy_partial(pr0)
                        pend_ent = []
                        _entropy_partial(r0)

                    # hidden mid-phase work
                    if ch == 5:
                        nc.gpsimd.collective_compute(
                            "ReduceScatter", A.add,
                            replica_groups=[list(range(N_CORES))],
                            ins=[cov_in.opt()], outs=[cov_out.opt()])
                    if ch == 13:
                        nc.sync.dma_start(covc[:], cov_out[:])
                        nc.vector.scalar_tensor_tensor(covc[:], rsb[:], rs_own[:],
                                                       covc[:], A.mult, A.add)
                        dtmp = hb_w.tile([128, 1024], F32, name="dtmp",
                                         tag="dt", bufs=2)
                        nc.vector.tensor_mul(dtmp[:], covc[:], dmask[:])
                        nc.vector.reduce_sum(d_own[:], dtmp[:], axis=AX)
                        nc.scalar.activation(d_own[:], d_own[:], AF.Sqrt)
                        nc.vector.reciprocal(rd_own[:], d_own[:])
                        nc.sync.dma_start(d_in[:], d_own[:])
                        nc.gpsimd.collective_compute(
                            "AllGather", A.bypass,
                            replica_groups=[list(range(N_CORES))],
                            ins=[d_in.opt()], outs=[d_out.opt()])
                    if ch == 16:
                        dcol = hb_w.tile([128, 8], F32, name="dcol", bufs=1)
                        nc.sync.dma_start(
                            dcol[:].rearrange("p (t o) -> p t o", o=1),
                            d_out[:].rearrange("(t p) c -> p t c", p=128))
                        rdc = hb_w.tile([128, 8], F32, name="rdc", bufs=1)
                        nc.vector.reciprocal(rdc[:], dcol[:])
                        _bcast_cols(nc, hb_w, hb_ps2, rdc, rdb, id128)
                    if ch == 18:
                        t1 = hb_w.tile([128, 1024], F32, name="t1", tag="dt", bufs=2)
                        nc.vector.tensor_mul(t1[:], covc[:], rdb[:])
                        nc.vector.tensor_single_scalar(t1[:], t1[:], rd_own[:],
                                                       A.mult)
                        t1c = hb_w.tile([128, 1024], F32, name="t1c", tag="dt", bufs=2)
                        nc.scalar.activation(t1c[:], t1[:], AF.Abs,
                                             accum_out=f1own[:])
                        nc.vector.tensor_single_scalar(f1own[:], f1own[:],
                                                       1.0 / float(B), A.mult)

            # ---------------- Phase C: entropies + mask ----------------------
            with (
                tc.tile_pool(name="pc_w", bufs=2) as pc_w,
                tc.tile_pool(name="pc_ps", bufs=2, space="PSUM") as pc_ps,
                tc.tile_pool(name="pc_io", bufs=6) as pc_io,
            ):
                # prefetch mask-phase loads (independent of pvec)
                xm_t = []
                um_t = []
                CH = 2048
                for c in range(D // CH):
                    xm = pc_io.tile([128, CH], F32, name="xm", tag="xm", bufs=3)
                    um = pc_io.tile([128, CH], F32, name="um", tag="um", bufs=3)
                    nc.sync.dma_start(xm[:], xr.ap()[:, c * CH:(c + 1) * CH])
                    nc.sync.dma_start(um[:], ur.ap()[:, c * CH:(c + 1) * CH])
                    xm_t.append(xm)
                    um_t.append(um)

                # batch-count partial + exchange
                bc_part = pc_w.tile([32, 32], F32, name="bc_part", bufs=1)
                nc.vector.reduce_sum(
                    bc_part[:],
                    counts_sb[:].rearrange("p (r l) -> p l r", r=RSL), axis=AX)
                nc.sync.dma_start(bc_in[:], bc_part[:])
                nc.gpsimd.collective_compute(
                    "AllGather", A.bypass,
                    replica_groups=[list(range(N_CORES))],
                    ins=[bc_in.opt()], outs=[bc_out.opt()])
                agb = pc_w.tile([32, 8, 32], F32, name="agb", bufs=1)
                nc.sync.dma_start(
                    agb[:], bc_out[:].rearrange("(r p) c -> p r c", p=32))
                bcs = pc_w.tile([32, 32], F32, name="bcs", bufs=1)
                nc.vector.reduce_sum(
                    bcs[:], agb[:].rearrange("p r c -> p c r"), axis=AX)
                lnb = pc_w.tile([32, 32], F32, name="lnb", bufs=1)
                nc.scalar.activation(lnb[:], bcs[:], AF.Ln,
                                     scale=scal[0:32, 13:14], bias=epsb[0:32, :])
                nc.vector.tensor_mul(lnb[:], lnb[:], bcs[:])
                sb1 = pc_w.tile([32, 1], F32, name="sb1", bufs=1)
                nc.vector.reduce_sum(sb1[:], lnb[:], axis=AX)
                nc.gpsimd.partition_all_reduce(sb1[:], sb1[:], 32, _rop("add"))
                hbat = pc_w.tile([128, 1], F32, name="hbat", bufs=1)
                nc.gpsimd.partition_broadcast(hbat[:], sb1[0:1, :])
                nc.vector.tensor_mul(hbat[:], hbat[:], scal[:, 15:16])

                # local entropies from accumulated partials
                psS = pc_ps.tile([1, RSL], F32, name="psS")
                nc.tensor.matmul(psS[:], ones32[:], erp_all[:], start=True,
                                 stop=True)
                srow = pc_w.tile([1, RSL], F32, name="srow", bufs=1)
                nc.scalar.copy(srow[:], psS[:])
                psT = pc_ps.tile([128, 1], F32, name="psT")
                nc.tensor.transpose(psT[:], srow[:], id128[:1, :1])
                hloc = pc_w.tile([128, 1], F32, name="hloc", bufs=1)
                nc.scalar.copy(hloc[:], psT[:])
                nc.vector.tensor_mul(hloc[:], hloc[:], scal[:, 14:15])

                # f2' = max(f2, 1/f2); keep = f1/f2'; p = 1-keep; rkeep = 1/keep
                tA = pc_w.tile([128, 1], F32, name="tA", bufs=1)
                tB = pc_w.tile([128, 1], F32, name="tB", bufs=1)
                nc.vector.reciprocal(tA[:], hbat[:])
                f2 = pc_w.tile([128, 1], F32, name="f2", bufs=1)
                nc.vector.tensor_mul(f2[:], hloc[:], tA[:])
                nc.vector.reciprocal(tB[:], f2[:])
                nc.vector.tensor_max(f2[:], f2[:], tB[:])
                nc.vector.reciprocal(tB[:], f2[:])
                nc.vector.tensor_mul(pvec[:, 2:3], f1own[:], tB[:])
                nc.vector.tensor_scalar(pvec[:, 0:1], pvec[:, 2:3], -1.0, 1.0,
                                        A.mult, A.add)
                nc.vector.reciprocal(pvec[:, 1:2], pvec[:, 2:3])

                if DEBUG:
                    dbg_sb = pc_w.tile([128, 16], F32, name="dbg_sb", bufs=1)
                    nc.scalar.copy(dbg_sb[:, 0:4], pvec[:])
                    nc.scalar.copy(dbg_sb[:, 4:5], hloc[:])
                    nc.scalar.copy(dbg_sb[:, 5:6], hbat[:])
                    nc.scalar.copy(dbg_sb[:, 6:14], scal[:, 4:12])
                    nc.scalar.copy(dbg_sb[:, 14:15], f1own[:])
                    nc.scalar.copy(dbg_sb[:, 15:16], d_own[:])
                    nc.sync.dma_start(dbg.ap(), dbg_sb[:])

                # mask + scale
                for c in range(D // CH):
                    xm, um = xm_t[c], um_t[c]
                    nc.vector.tensor_single_scalar(um[:], um[:], pvec[:, 0:1],
                                                   A.is_gt)
                    oc = pc_io.tile([128, CH], F32, name="oc", tag="oc", bufs=3)
                    nc.vector.scalar_tensor_tensor(oc[:], um[:], pvec[:, 1:2],
                                                   xm[:], A.mult, A.mult)
                    nc.sync.dma_start(out.ap()[:, c * CH:(c + 1) * CH], oc[:])

    nc.compile()
    return nc


def _rop(name):
    from concourse import bass_isa
    return getattr(bass_isa.ReduceOp, name)


def _bcast_cols(nc, sbuf_pool, psum_pool, vec8, dst, id128):
    """dst[p, t*128+q] = vec8[q, t]  (flatten [128,8] col-major, bcast to all
    partitions)."""
    import concourse.mybir as mybir
    F32 = mybir.dt.float32
    pt = psum_pool.tile([8, 128], F32, name="bc_pt")
    nc.tensor.transpose(pt[:8, :], vec8[:], id128[:])
    tr = sbuf_pool.tile([8, 128], F32, name="bc_tr", bufs=1)
    nc.scalar.copy(tr[:], pt[:8, :])
    flat = sbuf_pool.tile([1, 8 * 128], F32, name="bc_flat", bufs=1)
    for t in range(8):
        nc.sync.dma_start(flat[:, t * 128:(t + 1) * 128], tr[t:t + 1, :])
    nc.gpsimd.partition_broadcast(dst[:], flat[:])


def kernel(x, u):
    if "nc" not in _cache:
        _cache["nc"] = _build()
    nc = _cache["nc"]
    from concourse.bass_utils import run_bass_kernel_spmd

    x = np.asarray(x, dtype=np.float32)
    u = np.asarray(u, dtype=np.float32)
    orig_shape = x.shape
    xf = np.ascontiguousarray(x.reshape(B, D))
    uf = np.ascontiguousarray(u.reshape(B, D))
    in_maps = []
    for c in range(N_CORES):
        selv = np.zeros((128, 8), np.float32)
        selv[:, c] = 1.0
        in_maps.append({
            "xst": np.ascontiguousarray(xf[:, c * DSL:(c + 1) * DSL].T),
            "xr": np.ascontiguousarray(xf[c * RSL:(c + 1) * RSL, :]),
            "ur": np.ascontiguousarray(uf[c * RSL:(c + 1) * RSL, :]),
            "sel": selv,
        })
    res = run_bass_kernel_spmd(nc, in_maps, core_ids=list(range(N_CORES)))
    _cache["last_results"] = res
    outf = np.concatenate([res.results[c]["out"] for c in range(N_CORES)], axis=0)
    return outf.reshape(orig_shape)


# revision 10
# speedup vs baseline: 1.3847x; 1.0166x over previous
"""DifferentialDropout Trainium2 kernel (8-core SPMD), v3.

Reference semantics: per-row corrcoef factor f1, global-standardized 1000-bin
per-row histograms -> entropies -> per-row keep prob -> mask+scale.

Key design (vs 771.8 us baseline):
  - cov GEMM in float32r (1 cycle/row vs fp32's 4) with explicit rounding
    copies; rowsums via PE matmul on the same f32r tiles.
  - phase A stats computed from the xst (D-shard) tiles while they stream;
    ONE combined AllGather carries rowsum partials + min/max/sumsq, issued
    before the GEMM so phase B starts ~50 us.
  - histogram prep: fused floor chain (7 DVE ops/chunk) writing
    z2 = [hi | lo | hi-16 | lo-16]; ONE is_equal over z2 yields TWO planes
    (j, j+16) stored pair-interleaved (entropy is permutation-invariant
    over bins, so the scramble never needs undoing).
  - plane pairs split DVE/Act/Pool; Act uses Square+Relu pairs.
  - chunks 0-3 are Act-free (v on DVE, pairs redistributed, entropies
    deferred) so the GEMM's PSUM->SBUF staging copies can run on Act
    without head-blocking phase B; cov ReduceScatter issued at chunk 5,
    d AllGather at chunk 13 - all hidden under phase B.
  - mask-phase x/u loads prefetched before the batch-count exchange.
"""

import os
import sys

sys.path.insert(0, "/opt/trn_rl_repo")

import numpy as np

B = 1024
D = 16384
BINS = 1000
N_CORES = 8
DSL = D // N_CORES      # 2048
RSL = B // N_CORES      # 128
C_ROWS = 4
F = C_ROWS * 128        # 512 elems/partition/chunk
N_CH = RSL // C_ROWS    # 32 chunks
LN2 = 0.6931471805599453

# plane split: 16 pair-planes per chunk across engines
PR_DVE = int(os.environ.get("K_PRDVE", "11"))
PR_ACT = int(os.environ.get("K_PRACT", "2"))
PR_GPS = 16 - PR_DVE - PR_ACT
ACT_FREE_CH = int(os.environ.get("K_AFCH", "4"))   # Act-free leading chunks
EW_BUFS = int(os.environ.get("K_EWB", "6"))
DEBUG = os.environ.get("K_DEBUG", "0") == "1"

_cache = {}


def _build():
    import concourse.mybir as mybir
    import concourse.tile as tile
    from concourse import bacc
    from concourse.masks import make_identity

    F32 = mybir.dt.float32
    F32R = mybir.dt.float32r
    BF16 = mybir.dt.bfloat16
    I32 = mybir.dt.int32
    A = mybir.AluOpType
    AF = mybir.ActivationFunctionType
    AX = mybir.AxisListType.X

    nc = bacc.Bacc("TRN2", target_bir_lowering=False, debug=False,
                   num_devices=N_CORES)

    xst = nc.dram_tensor("xst", [DSL, B], F32, kind="ExternalInput")
    xr = nc.dram_tensor("xr", [RSL, D], F32, kind="ExternalInput")
    ur = nc.dram_tensor("ur", [RSL, D], F32, kind="ExternalInput")
    sel = nc.dram_tensor("sel", [128, 8], F32, kind="ExternalInput")
    out = nc.dram_tensor("out", [RSL, D], F32, kind="ExternalOutput")
    if DEBUG:
        dbg = nc.dram_tensor("dbg", [128, 16], F32, kind="ExternalOutput")

    xr_v = xr.ap().rearrange("r (p e) -> p r e", p=128)   # [128, 128, 128]

    with tile.TileContext(nc) as tc:
        with (
            tc.tile_pool(name="const", bufs=1) as constp,
            tc.tile_pool(name="persist", bufs=1) as persist,
            tc.tile_pool(name="dram", bufs=1, space="DRAM") as dram,
        ):
            id128 = constp.tile([128, 128], F32, name="id128")
            make_identity(nc, id128[:])
            ones32 = constp.tile([32, 1], F32, name="ones32")
            nc.vector.memset(ones32[:], 1.0)
            ones1f = constp.tile([128, 1], F32, name="ones1f")
            nc.vector.memset(ones1f[:], 1.0)
            ones1r = constp.tile([128, 1], F32R, name="ones1r")
            nc.vector.tensor_copy(ones1r[:], ones1f[:])
            epsb = constp.tile([128, 1], F32, name="epsb")
            nc.vector.memset(epsb[:], 1e-30)
            one1 = constp.tile([128, 1], F32, name="one1")
            nc.vector.memset(one1[:], 1.0)
            negq = constp.tile([128, 16], F32, name="negq")
            for _q in range(16):
                nc.vector.memset(negq[:, _q:_q + 1], -float(_q))

            # persistent SBUF
            counts_sb = persist.tile([32, RSL * 32], F32, name="counts_sb")
            erp_all = persist.tile([32, RSL], F32, name="erp_all")
            scal = persist.tile([128, 24], F32, name="scal")
            rsb = persist.tile([128, 1024], F32, name="rsb")
            rdb = persist.tile([128, 1024], F32, name="rdb")
            dmask = persist.tile([128, 1024], F32, name="dmask")
            covc = persist.tile([128, 1024], F32, name="covc")
            sel_sb = persist.tile([128, 8], F32, name="sel_sb")
            rs_own = persist.tile([128, 1], F32, name="rs_own")
            d_own = persist.tile([128, 1], F32, name="d_own")
            rd_own = persist.tile([128, 1], F32, name="rd_own")
            f1own = persist.tile([128, 1], F32, name="f1own")
            pvec = persist.tile([128, 4], F32, name="pvec")

            nc.sync.dma_start(sel_sb[:], sel.ap())

            # DRAM bounces
            ag_in = dram.tile([1, 1032], F32, name="ag_in")
            ag_out = dram.tile([8, 1032], F32, addr_space="Shared", name="ag_out")
            cov_in = dram.tile([1024, 1024], F32, name="cov_in")
            cov_out = dram.tile([128, 1024], F32, name="cov_out")
            d_in = dram.tile([128, 1], F32, name="d_in")
            d_out = dram.tile([1024, 1], F32, addr_space="Shared", name="d_out")
            bc_in = dram.tile([32, 32], F32, name="bc_in")
            bc_out = dram.tile([256, 32], F32, addr_space="Shared", name="bc_out")

            # ---------------- Phase A: stats + GEMM (f32r) -------------------
            with (
                tc.tile_pool(name="pa_io", bufs=3) as pa_io,
                tc.tile_pool(name="pa_big", bufs=1) as pa_big,
                tc.tile_pool(name="pa_w", bufs=2) as pa_w,
                tc.tile_pool(name="pa_ps2", bufs=2, space="PSUM") as pa_ps2,
                tc.tile_pool(name="pa_rs", bufs=1, space="PSUM") as pa_rs,
            ):
                xst_r = pa_big.tile([128, 16, 1024], F32R, name="xst_r")
                mn16 = pa_big.tile([128, 16], F32, name="mn16")
                mx16 = pa_big.tile([128, 16], F32, name="mx16")
                ssq16 = pa_big.tile([128, 16], F32, name="ssq16")
                rs_ps = pa_rs.tile([1, 2, 512], F32, name="rs_ps")
                for k in range(16):
                    xk = pa_io.tile([128, 1024], F32, name="xk")
                    nc.sync.dma_start(xk[:], xst.ap()[k * 128:(k + 1) * 128, :])
                    # f32r rounding copy on Pool; min+max reduces on DVE
                    nc.gpsimd.tensor_copy(xst_r[:, k, :], xk[:])
                    nc.vector.reduce_sum(mn16[:, k:k + 1], xk[:], axis=AX, op=A.min)
                    nc.vector.reduce_sum(mx16[:, k:k + 1], xk[:], axis=AX, op=A.max)
                    sq = pa_w.tile([128, 1024], F32, name="sq", tag="sq", bufs=2)
                    nc.scalar.activation(sq[:], xk[:], AF.Square,
                                         accum_out=ssq16[:, k:k + 1])
                    # rowsum partial via PE (accumulate over k)
                    nc.tensor.matmul(rs_ps[:, 0, :], ones1r[:], xst_r[:, k, 0:512],
                                     start=(k == 0), stop=(k == 15))
                    nc.tensor.matmul(rs_ps[:, 1, :], ones1r[:], xst_r[:, k, 512:1024],
                                     start=(k == 0), stop=(k == 15))

                # local reductions -> per-partition partials
                pmn = pa_w.tile([128, 1], F32, name="pmn", bufs=1)
                nc.vector.reduce_sum(pmn[:], mn16[:], axis=AX, op=A.min)
                nc.vector.tensor_single_scalar(pmn[:], pmn[:], -1.0, A.mult)
                pmx = pa_w.tile([128, 1], F32, name="pmx", bufs=1)
                nc.vector.reduce_sum(pmx[:], mx16[:], axis=AX, op=A.max)
                pss = pa_w.tile([128, 1], F32, name="pss", bufs=1)
                nc.vector.reduce_sum(pss[:], ssq16[:], axis=AX)
                # cross-partition (128) reduce
                nc.gpsimd.partition_all_reduce(pmn[:], pmn[:], 128, _rop("max"))
                nc.gpsimd.partition_all_reduce(pmx[:], pmx[:], 128, _rop("max"))
                nc.gpsimd.partition_all_reduce(pss[:], pss[:], 128, _rop("add"))
                rs_sb = pa_w.tile([1, 1024], F32, name="rs_sb", bufs=1)
                nc.scalar.copy(rs_sb[:, 0:512], rs_ps[:, 0, :])
                nc.scalar.copy(rs_sb[:, 512:1024], rs_ps[:, 1, :])
                nc.sync.dma_start(ag_in[:, 0:1024], rs_sb[:])
                nc.sync.dma_start(ag_in[:, 1024:1025], pmn[0:1, :])
                nc.sync.dma_start(ag_in[:, 1025:1026], pmx[0:1, :])
                nc.sync.dma_start(ag_in[:, 1026:1027], pss[0:1, :])
                nc.gpsimd.collective_compute(
                    "AllGather", A.bypass,
                    replica_groups=[list(range(N_CORES))],
                    ins=[ag_in.opt()], outs=[ag_out.opt()])

                # -------- process gathered stats --------
                agsb = pa_w.tile([8, 1032], F32, name="agsb", bufs=1)
                nc.sync.dma_start(agsb[:], ag_out[:])
                rsfull = pa_w.tile([8, 1024], F32, name="rsfull", bufs=1)
                nc.gpsimd.partition_all_reduce(rsfull[:], agsb[:, 0:1024], 8,
                                               _rop("add"))
                g3 = pa_w.tile([8, 3], F32, name="g3", bufs=1)
                nc.gpsimd.partition_all_reduce(g3[:, 0:1], agsb[:, 1024:1025], 8,
                                               _rop("max"))
                nc.gpsimd.partition_all_reduce(g3[:, 1:2], agsb[:, 1025:1026], 8,
                                               _rop("max"))
                nc.gpsimd.partition_all_reduce(g3[:, 2:3], agsb[:, 1026:1027], 8,
                                               _rop("add"))
                nc.gpsimd.partition_broadcast(rsb[:], rsfull[0:1, :])
                nc.gpsimd.partition_broadcast(scal[:, 2:3], g3[0:1, 0:1])  # -min
                nc.gpsimd.partition_broadcast(scal[:, 3:4], g3[0:1, 1:2])  # max
                nc.gpsimd.partition_broadcast(scal[:, 1:2], g3[0:1, 2:3])  # ssq
                nc.vector.reduce_sum(scal[:, 0:1], rsb[:], axis=AX)        # gsum

                # -------- scalar constants (same math as baseline) --------
                N_f = float(B) * float(D)
                nc.vector.tensor_single_scalar(scal[:, 4:5], scal[:, 0:1], 1.0 / N_f, A.mult)
                nc.vector.tensor_mul(scal[:, 20:21], scal[:, 0:1], scal[:, 4:5])
                nc.vector.tensor_sub(scal[:, 20:21], scal[:, 1:2], scal[:, 20:21])
                nc.vector.tensor_single_scalar(scal[:, 20:21], scal[:, 20:21],
                                               1.0 / (N_f - 1.0), A.mult)
                nc.scalar.activation(scal[:, 5:6], scal[:, 20:21], AF.Sqrt)
                nc.vector.reciprocal(scal[:, 6:7], scal[:, 5:6])
                nc.vector.tensor_single_scalar(scal[:, 21:22], scal[:, 2:3], -1.0, A.mult)
                nc.vector.tensor_sub(scal[:, 21:22], scal[:, 21:22], scal[:, 4:5])
                nc.vector.tensor_mul(scal[:, 7:8], scal[:, 21:22], scal[:, 6:7])
                nc.vector.tensor_sub(scal[:, 22:23], scal[:, 3:4], scal[:, 4:5])
                nc.vector.tensor_mul(scal[:, 22:23], scal[:, 22:23], scal[:, 6:7])
                nc.vector.tensor_sub(scal[:, 22:23], scal[:, 22:23], scal[:, 7:8])
                nc.vector.tensor_single_scalar(scal[:, 8:9], scal[:, 22:23],
                                               1.0 / BINS, A.mult)
                nc.vector.reciprocal(scal[:, 9:10], scal[:, 8:9])
                nc.vector.tensor_mul(scal[:, 10:11], scal[:, 6:7], scal[:, 9:10])
                nc.vector.tensor_mul(scal[:, 23:24], scal[:, 4:5], scal[:, 6:7])
                nc.vector.tensor_add(scal[:, 23:24], scal[:, 23:24], scal[:, 7:8])
                nc.vector.tensor_mul(scal[:, 23:24], scal[:, 23:24], scal[:, 9:10])
                nc.vector.tensor_single_scalar(scal[:, 11:12], scal[:, 23:24], -1.0, A.mult)
                nc.vector.tensor_single_scalar(scal[:, 16:17], scal[:, 8:9], float(D), A.mult)
                nc.vector.reciprocal(scal[:, 12:13], scal[:, 16:17])
                nc.vector.tensor_single_scalar(scal[:, 17:18], scal[:, 8:9], N_f, A.mult)
                nc.vector.reciprocal(scal[:, 13:14], scal[:, 17:18])
                nc.vector.tensor_single_scalar(scal[:, 14:15], scal[:, 12:13],
                                               -1.0 / LN2, A.mult)
                nc.vector.tensor_single_scalar(scal[:, 15:16], scal[:, 13:14],
                                               -1.0 / LN2, A.mult)

                # dmask[p, t*128+q] = sel[p, t] * (q == p); rs_own
                for t in range(8):
                    nc.vector.tensor_single_scalar(
                        dmask[:, t * 128:(t + 1) * 128], id128[:],
                        sel_sb[:, t:t + 1], A.mult)
                rtmp = pa_w.tile([128, 1024], F32, name="rtmp", bufs=1)
                nc.vector.tensor_mul(rtmp[:], rsb[:], dmask[:])
                nc.vector.reduce_sum(rs_own[:], rtmp[:], axis=AX)
                nc.vector.tensor_single_scalar(rs_own[:], rs_own[:],
                                               -1.0 / float(D), A.mult)

                # GEMM: covp = xs @ xs.T (K-sharded partial), f32r.
                # Per-block staging on Act (phase B chunks 0..3 are Act-free)
                # with immediate per-block DMA into the cov_in bounce, so the
                # ReduceScatter input is ready right at GEMM end and phase A's
                # SBUF footprint stays small.
                cov_in_v = cov_in[:].rearrange("(m p) j -> p m j", p=128)
                for m in range(8):
                    for n2 in range(2):
                        pg = pa_ps2.tile([128, 512], F32, name="pg")
                        for k in range(16):
                            nc.tensor.matmul(
                                pg[:],
                                xst_r[:, k, m * 128:(m + 1) * 128],
                                xst_r[:, k, n2 * 512:(n2 + 1) * 512],
                                start=(k == 0), stop=(k == 15))
                        cvb = pa_w.tile([128, 512], F32, name="cvb", tag="cvb",
                                        bufs=2)
                        nc.scalar.copy(cvb[:], pg[:])
                        nc.sync.dma_start(
                            cov_in_v[:, m, n2 * 512:(n2 + 1) * 512], cvb[:])

            # ---------------- Phase B: histogram ----------------------------
            with (
                tc.tile_pool(name="hb_io", bufs=3) as hb_io,
                tc.tile_pool(name="hb_w", bufs=2) as hb_w,
                tc.tile_pool(name="hb_z", bufs=2) as hb_z,
                tc.tile_pool(name="hb_pl", bufs=2) as hb_pl,
                tc.tile_pool(name="hb_ps", bufs=6, space="PSUM") as hb_ps,
                tc.tile_pool(name="hb_ps2", bufs=1, space="PSUM") as hb_ps2,
            ):
                pend_ent = []

                def _entropy_partial(er0):
                    lnc = hb_w.tile([32, C_ROWS * 32], F32, name="lnc",
                                    tag="ln", bufs=2)
                    nc.scalar.activation(
                        lnc[:], counts_sb[:, er0 * 32:(er0 + C_ROWS) * 32],
                        AF.Ln, scale=scal[0:32, 12:13], bias=epsb[0:32, :])
                    nc.vector.tensor_mul(
                        lnc[:], lnc[:], counts_sb[:, er0 * 32:(er0 + C_ROWS) * 32])
                    nc.vector.reduce_sum(
                        erp_all[:, er0:er0 + C_ROWS],
                        lnc[:].rearrange("p (r l) -> p r l", r=C_ROWS), axis=AX)

                for ch in range(N_CH):
                    act_free = ch < ACT_FREE_CH
                    r0 = ch * C_ROWS
                    xch = hb_io.tile([128, C_ROWS, 128], F32, name="xch")
                    nc.sync.dma_start(xch[:], xr_v[:, r0:r0 + C_ROWS, :])
                    xf = xch[:].rearrange("p a b -> p (a b)")

                    # v = SC*x + BC (Act: +Relu; DVE fallback lets the rare
                    # v<0 element drop from the histogram, which is harmless)
                    v = hb_w.tile([128, F], F32, name="v", tag="ew", bufs=EW_BUFS)
                    if act_free:
                        nc.vector.tensor_scalar(v[:], xf, scal[:, 10:11],
                                                scal[:, 11:12], A.mult, A.add)
                    else:
                        nc.scalar.activation(v[:], xf, AF.Relu,
                                             bias=scal[:, 11:12],
                                             scale=scal[:, 10:11])
                    i1 = hb_w.tile([128, F], I32, name="i1", tag="ew", bufs=EW_BUFS)
                    nc.vector.tensor_single_scalar(i1[:], v[:], 999.49, A.min)
                    g1 = hb_w.tile([128, F], F32, name="g1", tag="ew", bufs=EW_BUFS)
                    nc.vector.tensor_tensor(g1[:], i1[:], v[:], A.is_gt)
                    idxf = hb_w.tile([128, F], F32, name="idxf", tag="ew", bufs=EW_BUFS)
                    nc.vector.tensor_tensor(idxf[:], i1[:], g1[:], A.subtract)
                    ihi = hb_w.tile([128, F], I32, name="ihi", tag="ew", bufs=EW_BUFS)
                    nc.vector.tensor_scalar(ihi[:], idxf[:], 1.0 / 32.0, -0.484375,
                                            A.mult, A.add)
                    z2 = hb_z.tile([128, 4 * F], BF16, name="z2")
                    nc.vector.tensor_copy(z2[:, 0:F], ihi[:])
                    nc.vector.scalar_tensor_tensor(z2[:, F:2 * F], ihi[:], -32.0,
                                                   idxf[:], A.mult, A.add)
                    nc.vector.tensor_single_scalar(z2[:, 2 * F:4 * F],
                                                   z2[:, 0:2 * F], 16.0,
                                                   A.subtract)

                    # pair planes: one is_equal over z2=[z | z-16] yields planes
                    # (j, j+16) stored interleaved at slots (2j, 2j+1).  All
                    # downstream uses of counts are permutation-invariant.
                    P = hb_pl.tile([128, 32, 2 * F], BF16, name="P")
                    Pv = P[:].rearrange("p q f -> p (q f)")
                    n_dve = PR_DVE + (1 if act_free else 0)
                    n_gps = PR_GPS + (1 if act_free else 0)
                    for j in range(n_dve):
                        nc.vector.tensor_single_scalar(
                            Pv[:, j * 4 * F:(j + 1) * 4 * F], z2[:], float(j),
                            A.is_equal)
                    for j in range(n_dve, n_dve + n_gps):
                        nc.gpsimd.tensor_single_scalar(
                            Pv[:, j * 4 * F:(j + 1) * 4 * F], z2[:], float(j),
                            A.is_equal)
                    for j in range(n_dve + n_gps, 16):
                        atmp = hb_w.tile([128, 4 * F], BF16, name="atmp",
                                         tag="at", bufs=1)
                        nc.scalar.activation(atmp[:], z2[:], AF.Square,
                                             bias=negq[:, j:j + 1])
                        nc.scalar.activation(Pv[:, j * 4 * F:(j + 1) * 4 * F],
                                             atmp[:], AF.Relu,
                                             bias=one1[:], scale=-1.0)

                    for r in range(C_ROWS):
                        ps = hb_ps.tile([32, 32], F32, name="ps")
                        for k in range(128):
                            t = r * 128 + k
                            nc.tensor.matmul(ps[:], P[:, :, t], P[:, :, F + t],
                                             start=(k == 0), stop=(k == 127))
                        rr = r0 + r
                        if act_free:
                            nc.vector.tensor_copy(
                                counts_sb[:, rr * 32:(rr + 1) * 32], ps[:32, :])
                        else:
                            nc.scalar.copy(
                                counts_sb[:, rr * 32:(rr + 1) * 32], ps[:32, :])

                    # per-chunk local-entropy partials (deferred while Act-free)
                    if act_free:
                        pend_ent.append(r0)
                    else:
                        for pr0 in pend_ent:
                            _entropy_partial(pr0)
                        pend_ent = []
                        _entropy_partial(r0)

                    # hidden mid-phase work
                    if ch == 5:
                        nc.gpsimd.collective_compute(
                            "ReduceScatter", A.add,
                            replica_groups=[list(range(N_CORES))],
                            ins=[cov_in.opt()], outs=[cov_out.opt()])
                    if ch == 13:
                        nc.sync.dma_start(covc[:], cov_out[:])
                        nc.vector.scalar_tensor_tensor(covc[:], rsb[:], rs_own[:],
                                                       covc[:], A.mult, A.add)
                        dtmp = hb_w.tile([128, 1024], F32, name="dtmp",
                                         tag="dt", bufs=2)
                        nc.vector.tensor_mul(dtmp[:], covc[:], dmask[:])
                        nc.vector.reduce_sum(d_own[:], dtmp[:], axis=AX)
                        nc.scalar.activation(d_own[:], d_own[:], AF.Sqrt)
                        nc.vector.reciprocal(rd_own[:], d_own[:])
                        nc.sync.dma_start(d_in[:], d_own[:])
                        nc.gpsimd.collective_compute(
                            "AllGather", A.bypass,
                            replica_groups=[list(range(N_CORES))],
                            ins=[d_in.opt()], outs=[d_out.opt()])
                    if ch == 16:
                        dcol = hb_w.tile([128, 8], F32, name="dcol", bufs=1)
                        nc.sync.dma_start(
                            dcol[:].rearrange("p (t o) -> p t o", o=1),
                            d_out[:].rearrange("(t p) c -> p t c", p=128))
                        rdc = hb_w.tile([128, 8], F32, name="rdc", bufs=1)
                        nc.vector.reciprocal(rdc[:], dcol[:])
                        _bcast_cols(nc, hb_w, hb_ps2, rdc, rdb, id128)
                    if ch == 18:
                        t1 = hb_w.tile([128, 1024], F32, name="t1", tag="dt", bufs=2)
                        nc.vector.tensor_mul(t1[:], covc[:], rdb[:])
                        nc.vector.tensor_single_scalar(t1[:], t1[:], rd_own[:],
                                                       A.mult)
                        t1c = hb_w.tile([128, 1024], F32, name="t1c", tag="dt", bufs=2)
                        nc.scalar.activation(t1c[:], t1[:], AF.Abs,
                                             accum_out=f1own[:])
                        nc.vector.tensor_single_scalar(f1own[:], f1own[:],
                                                       1.0 / float(B), A.mult)

            # ---------------- Phase C: entropies + mask ----------------------
            with (
                tc.tile_pool(name="pc_w", bufs=2) as pc_w,
                tc.tile_pool(name="pc_ps", bufs=2, space="PSUM") as pc_ps,
                tc.tile_pool(name="pc_io", bufs=6) as pc_io,
            ):
                # prefetch mask-phase loads (independent of pvec)
                xm_t = []
                um_t = []
                CH = 2048
                for c in range(D // CH):
                    xm = pc_io.tile([128, CH], F32, name="xm", tag="xm", bufs=3)
                    um = pc_io.tile([128, CH], F32, name="um", tag="um", bufs=3)
                    nc.sync.dma_start(xm[:], xr.ap()[:, c * CH:(c + 1) * CH])
                    nc.sync.dma_start(um[:], ur.ap()[:, c * CH:(c + 1) * CH])
                    xm_t.append(xm)
                    um_t.append(um)

                # batch-count partial + exchange
                bc_part = pc_w.tile([32, 32], F32, name="bc_part", bufs=1)
                nc.vector.reduce_sum(
                    bc_part[:],
                    counts_sb[:].rearrange("p (r l) -> p l r", r=RSL), axis=AX)
                nc.sync.dma_start(bc_in[:], bc_part[:])
                nc.gpsimd.collective_compute(
                    "AllGather", A.bypass,
                    replica_groups=[list(range(N_CORES))],
                    ins=[bc_in.opt()], outs=[bc_out.opt()])

                # local entropies from accumulated partials (hidden under AG)
                psS = pc_ps.tile([1, RSL], F32, name="psS")
                nc.tensor.matmul(psS[:], ones32[:], erp_all[:], start=True,
                                 stop=True)
                srow = pc_w.tile([1, RSL], F32, name="srow", bufs=1)
                nc.scalar.copy(srow[:], psS[:])
                psT = pc_ps.tile([128, 1], F32, name="psT")
                nc.tensor.transpose(psT[:], srow[:], id128[:1, :1])
                hloc = pc_w.tile([128, 1], F32, name="hloc", bufs=1)
                nc.scalar.copy(hloc[:], psT[:])
                nc.vector.tensor_mul(hloc[:], hloc[:], scal[:, 14:15])

                agb = pc_w.tile([32, 8, 32], F32, name="agb", bufs=1)
                nc.sync.dma_start(
                    agb[:], bc_out[:].rearrange("(r p) c -> p r c", p=32))
                bcs = pc_w.tile([32, 32], F32, name="bcs", bufs=1)
                nc.vector.reduce_sum(
                    bcs[:], agb[:].rearrange("p r c -> p c r"), axis=AX)
                lnb = pc_w.tile([32, 32], F32, name="lnb", bufs=1)
                nc.scalar.activation(lnb[:], bcs[:], AF.Ln,
                                     scale=scal[0:32, 13:14], bias=epsb[0:32, :])
                nc.vector.tensor_mul(lnb[:], lnb[:], bcs[:])
                sb1 = pc_w.tile([32, 1], F32, name="sb1", bufs=1)
                nc.vector.reduce_sum(sb1[:], lnb[:], axis=AX)
                nc.gpsimd.partition_all_reduce(sb1[:], sb1[:], 32, _rop("add"))
                hbat = pc_w.tile([128, 1], F32, name="hbat", bufs=1)
                nc.gpsimd.partition_broadcast(hbat[:], sb1[0:1, :])
                nc.vector.tensor_mul(hbat[:], hbat[:], scal[:, 15:16])

                # f2' = max(f2, 1/f2); keep = f1/f2'; p = 1-keep; rkeep = 1/keep
                tA = pc_w.tile([128, 1], F32, name="tA", bufs=1)
                tB = pc_w.tile([128, 1], F32, name="tB", bufs=1)
                nc.vector.reciprocal(tA[:], hbat[:])
                f2 = pc_w.tile([128, 1], F32, name="f2", bufs=1)
                nc.vector.tensor_mul(f2[:], hloc[:], tA[:])
                nc.vector.reciprocal(tB[:], f2[:])
                nc.vector.tensor_max(f2[:], f2[:], tB[:])
                nc.vector.reciprocal(tB[:], f2[:])
                nc.vector.tensor_mul(pvec[:, 2:3], f1own[:], tB[:])
                nc.vector.tensor_scalar(pvec[:, 0:1], pvec[:, 2:3], -1.0, 1.0,
                                        A.mult, A.add)
                nc.vector.reciprocal(pvec[:, 1:2], pvec[:, 2:3])

                if DEBUG:
                    dbg_sb = pc_w.tile([128, 16], F32, name="dbg_sb", bufs=1)
                    nc.scalar.copy(dbg_sb[:, 0:4], pvec[:])
                    nc.scalar.copy(dbg_sb[:, 4:5], hloc[:])
                    nc.scalar.copy(dbg_sb[:, 5:6], hbat[:])
                    nc.scalar.copy(dbg_sb[:, 6:14], scal[:, 4:12])
                    nc.scalar.copy(dbg_sb[:, 14:15], f1own[:])
                    nc.scalar.copy(dbg_sb[:, 15:16], d_own[:])
                    nc.sync.dma_start(dbg.ap(), dbg_sb[:])

                # mask + scale
                for c in range(D // CH):
                    xm, um = xm_t[c], um_t[c]
                    nc.vector.tensor_single_scalar(um[:], um[:], pvec[:, 0:1],
                                                   A.is_gt)
                    oc = pc_io.tile([128, CH], F32, name="oc", tag="oc", bufs=3)
                    nc.vector.scalar_tensor_tensor(oc[:], um[:], pvec[:, 1:2],
                                                   xm[:], A.mult, A.mult)
                    nc.sync.dma_start(out.ap()[:, c * CH:(c + 1) * CH], oc[:])

    nc.compile()
    return nc


def _rop(name):
    from concourse import bass_isa
    return getattr(bass_isa.ReduceOp, name)


def _bcast_cols(nc, sbuf_pool, psum_pool, vec8, dst, id128):
    """dst[p, t*128+q] = vec8[q, t]  (flatten [128,8] col-major, bcast to all
    partitions)."""
    import concourse.mybir as mybir
    F32 = mybir.dt.float32
    pt = psum_pool.tile([8, 128], F32, name="bc_pt")
    nc.tensor.transpose(pt[:8, :], vec8[:], id128[:])
    tr = sbuf_pool.tile([8, 128], F32, name="bc_tr", bufs=1)
    nc.scalar.copy(tr[:], pt[:8, :])
    flat = sbuf_pool.tile([1, 8 * 128], F32, name="bc_flat", bufs=1)
    for t in range(8):
        nc.sync.dma_start(flat[:, t * 128:(t + 1) * 128], tr[t:t + 1, :])
    nc.gpsimd.partition_broadcast(dst[:], flat[:])


def kernel(x, u):
    if "nc" not in _cache:
        _cache["nc"] = _build()
    nc = _cache["nc"]
    from concourse.bass_utils import run_bass_kernel_spmd

    x = np.asarray(x, dtype=np.float32)
    u = np.asarray(u, dtype=np.float32)
    orig_shape = x.shape
    xf = np.ascontiguousarray(x.reshape(B, D))
    uf = np.ascontiguousarray(u.reshape(B, D))
    in_maps = []
    for c in range(N_CORES):
        selv = np.zeros((128, 8), np.float32)
        selv[:, c] = 1.0
        in_maps.append({
            "xst": np.ascontiguousarray(xf[:, c * DSL:(c + 1) * DSL].T),
            "xr": np.ascontiguousarray(xf[c * RSL:(c + 1) * RSL, :]),
            "ur": np.ascontiguousarray(uf[c * RSL:(c + 1) * RSL, :]),
            "sel": selv,
        })
    res = run_bass_kernel_spmd(nc, in_maps, core_ids=list(range(N_CORES)))
    _cache["last_results"] = res
    outf = np.concatenate([res.results[c]["out"] for c in range(N_CORES)], axis=0)
    return outf.reshape(orig_shape)


# revision 13
# speedup vs baseline: 1.3892x; 1.0033x over previous
"""DifferentialDropout Trainium2 kernel (8-core SPMD), v3.

Reference semantics: per-row corrcoef factor f1, global-standardized 1000-bin
per-row histograms -> entropies -> per-row keep prob -> mask+scale.

Key design (vs 771.8 us baseline):
  - cov GEMM in float32r (1 cycle/row vs fp32's 4) with explicit rounding
    copies; rowsums via PE matmul on the same f32r tiles.
  - phase A stats computed from the xst (D-shard) tiles while they stream;
    ONE combined AllGather carries rowsum partials + min/max/sumsq, issued
    before the GEMM so phase B starts ~50 us.
  - histogram prep: fused floor chain (7 DVE ops/chunk) writing
    z2 = [hi | lo | hi-16 | lo-16]; ONE is_equal over z2 yields TWO planes
    (j, j+16) stored pair-interleaved (entropy is permutation-invariant
    over bins, so the scramble never needs undoing).
  - plane pairs split DVE/Act/Pool; Act uses Square+Relu pairs.
  - chunks 0-3 are Act-free (v on DVE, pairs redistributed, entropies
    deferred) so the GEMM's PSUM->SBUF staging copies can run on Act
    without head-blocking phase B; cov ReduceScatter issued at chunk 5,
    d AllGather at chunk 13 - all hidden under phase B.
  - mask-phase x/u loads prefetched before the batch-count exchange.
"""

import os
import sys

sys.path.insert(0, "/opt/trn_rl_repo")

import numpy as np

B = 1024
D = 16384
BINS = 1000
N_CORES = 8
DSL = D // N_CORES      # 2048
RSL = B // N_CORES      # 128
C_ROWS = 4
F = C_ROWS * 128        # 512 elems/partition/chunk
N_CH = RSL // C_ROWS    # 32 chunks
LN2 = 0.6931471805599453

# plane split: 16 pair-planes per chunk across engines
PR_DVE = int(os.environ.get("K_PRDVE", "11"))
PR_ACT = int(os.environ.get("K_PRACT", "2"))
PR_GPS = 16 - PR_DVE - PR_ACT
ACT_FREE_CH = int(os.environ.get("K_AFCH", "4"))   # Act-free leading chunks
EW_BUFS = int(os.environ.get("K_EWB", "6"))
DEBUG = os.environ.get("K_DEBUG", "0") == "1"

_cache = {}


def _build():
    import concourse.mybir as mybir
    import concourse.tile as tile
    from concourse import bacc
    from concourse.masks import make_identity

    F32 = mybir.dt.float32
    F32R = mybir.dt.float32r
    BF16 = mybir.dt.bfloat16
    I32 = mybir.dt.int32
    A = mybir.AluOpType
    AF = mybir.ActivationFunctionType
    AX = mybir.AxisListType.X

    nc = bacc.Bacc("TRN2", target_bir_lowering=False, debug=False,
                   num_devices=N_CORES)

    xst = nc.dram_tensor("xst", [DSL, B], F32, kind="ExternalInput")
    xr = nc.dram_tensor("xr", [RSL, D], F32, kind="ExternalInput")
    ur = nc.dram_tensor("ur", [RSL, D], F32, kind="ExternalInput")
    sel = nc.dram_tensor("sel", [128, 8], F32, kind="ExternalInput")
    out = nc.dram_tensor("out", [RSL, D], F32, kind="ExternalOutput")
    if DEBUG:
        dbg = nc.dram_tensor("dbg", [128, 16], F32, kind="ExternalOutput")

    xr_v = xr.ap().rearrange("r (p e) -> p r e", p=128)   # [128, 128, 128]

    with tile.TileContext(nc) as tc:
        with (
            tc.tile_pool(name="const", bufs=1) as constp,
            tc.tile_pool(name="persist", bufs=1) as persist,
            tc.tile_pool(name="dram", bufs=1, space="DRAM") as dram,
        ):
            id128 = constp.tile([128, 128], F32, name="id128")
            make_identity(nc, id128[:])
            ones32 = constp.tile([32, 1], F32, name="ones32")
            nc.vector.memset(ones32[:], 1.0)
            ones1f = constp.tile([128, 1], F32, name="ones1f")
            nc.vector.memset(ones1f[:], 1.0)
            ones1r = constp.tile([128, 1], F32R, name="ones1r")
            nc.vector.tensor_copy(ones1r[:], ones1f[:])
            epsb = constp.tile([128, 1], F32, name="epsb")
            nc.vector.memset(epsb[:], 1e-30)
            one1 = constp.tile([128, 1], F32, name="one1")
            nc.vector.memset(one1[:], 1.0)
            negq = constp.tile([128, 16], F32, name="negq")
            for _q in range(16):
                nc.vector.memset(negq[:, _q:_q + 1], -float(_q))

            # persistent SBUF
            counts_sb = persist.tile([32, RSL * 32], F32, name="counts_sb")
            erp_all = persist.tile([32, RSL], F32, name="erp_all")
            scal = persist.tile([128, 24], F32, name="scal")
            rsb = persist.tile([128, 1024], F32, name="rsb")
            rdb = persist.tile([128, 1024], F32, name="rdb")
            dmask = persist.tile([128, 1024], F32, name="dmask")
            covc = persist.tile([128, 1024], F32, name="covc")
            sel_sb = persist.tile([128, 8], F32, name="sel_sb")
            rs_own = persist.tile([128, 1], F32, name="rs_own")
            rd_own = persist.tile([128, 1], F32, name="rd_own")
            f1own = persist.tile([128, 1], F32, name="f1own")
            pvec = persist.tile([128, 4], F32, name="pvec")

            nc.sync.dma_start(sel_sb[:], sel.ap())

            # DRAM bounces
            ag_in = dram.tile([1, 2056], F32, name="ag_in")
            ag_out = dram.tile([8, 2056], F32, addr_space="Shared", name="ag_out")
            cov_in = dram.tile([1024, 1024], F32, name="cov_in")
            cov_out = dram.tile([128, 1024], F32, name="cov_out")
            bc_in = dram.tile([32, 32], F32, name="bc_in")
            bc_out = dram.tile([256, 32], F32, addr_space="Shared", name="bc_out")

            # ---------------- Phase A: stats + GEMM (f32r) -------------------
            with (
                tc.tile_pool(name="pa_io", bufs=3) as pa_io,
                tc.tile_pool(name="pa_big", bufs=1) as pa_big,
                tc.tile_pool(name="pa_w", bufs=2) as pa_w,
                tc.tile_pool(name="pa_ps2", bufs=2, space="PSUM") as pa_ps2,
                tc.tile_pool(name="pa_rs", bufs=1, space="PSUM") as pa_rs,
            ):
                xst_r = pa_big.tile([128, 16, 1024], F32R, name="xst_r")
                mn16 = pa_big.tile([128, 16], F32, name="mn16")
                mx16 = pa_big.tile([128, 16], F32, name="mx16")
                ssq16 = pa_big.tile([128, 16], F32, name="ssq16")
                rs_ps = pa_rs.tile([1, 2, 512], F32, name="rs_ps")
                rs2_ps = pa_rs.tile([1, 2, 512], F32, name="rs2_ps")
                for k in range(16):
                    xk = pa_io.tile([128, 1024], F32, name="xk")
                    nc.sync.dma_start(xk[:], xst.ap()[k * 128:(k + 1) * 128, :])
                    # f32r rounding copy on Pool; min+max reduces on DVE
                    nc.gpsimd.tensor_copy(xst_r[:, k, :], xk[:])
                    nc.vector.reduce_sum(mn16[:, k:k + 1], xk[:], axis=AX, op=A.min)
                    nc.vector.reduce_sum(mx16[:, k:k + 1], xk[:], axis=AX, op=A.max)
                    sq = pa_w.tile([128, 1024], F32R, name="sq", tag="sq", bufs=2)
                    nc.scalar.activation(sq[:], xk[:], AF.Square,
                                         accum_out=ssq16[:, k:k + 1])
                    # rowsum + row-sumsq partials via PE (accumulate over k)
                    nc.tensor.matmul(rs_ps[:, 0, :], ones1r[:], xst_r[:, k, 0:512],
                                     start=(k == 0), stop=(k == 15))
                    nc.tensor.matmul(rs_ps[:, 1, :], ones1r[:], xst_r[:, k, 512:1024],
                                     start=(k == 0), stop=(k == 15))
                    nc.tensor.matmul(rs2_ps[:, 0, :], ones1r[:], sq[:, 0:512],
                                     start=(k == 0), stop=(k == 15))
                    nc.tensor.matmul(rs2_ps[:, 1, :], ones1r[:], sq[:, 512:1024],
                                     start=(k == 0), stop=(k == 15))

                # local reductions -> per-partition partials
                pmn = pa_w.tile([128, 1], F32, name="pmn", bufs=1)
                nc.vector.reduce_sum(pmn[:], mn16[:], axis=AX, op=A.min)
                nc.vector.tensor_single_scalar(pmn[:], pmn[:], -1.0, A.mult)
                pmx = pa_w.tile([128, 1], F32, name="pmx", bufs=1)
                nc.vector.reduce_sum(pmx[:], mx16[:], axis=AX, op=A.max)
                pss = pa_w.tile([128, 1], F32, name="pss", bufs=1)
                nc.vector.reduce_sum(pss[:], ssq16[:], axis=AX)
                # cross-partition (128) reduce
                nc.gpsimd.partition_all_reduce(pmn[:], pmn[:], 128, _rop("max"))
                nc.gpsimd.partition_all_reduce(pmx[:], pmx[:], 128, _rop("max"))
                nc.gpsimd.partition_all_reduce(pss[:], pss[:], 128, _rop("add"))
                rs_sb = pa_w.tile([1, 2048], F32, name="rs_sb", bufs=1)
                nc.scalar.copy(rs_sb[:, 0:512], rs_ps[:, 0, :])
                nc.scalar.copy(rs_sb[:, 512:1024], rs_ps[:, 1, :])
                nc.scalar.copy(rs_sb[:, 1024:1536], rs2_ps[:, 0, :])
                nc.scalar.copy(rs_sb[:, 1536:2048], rs2_ps[:, 1, :])
                nc.sync.dma_start(ag_in[:, 0:2048], rs_sb[:])
                nc.sync.dma_start(ag_in[:, 2048:2049], pmn[0:1, :])
                nc.sync.dma_start(ag_in[:, 2049:2050], pmx[0:1, :])
                nc.sync.dma_start(ag_in[:, 2050:2051], pss[0:1, :])
                nc.gpsimd.collective_compute(
                    "AllGather", A.bypass,
                    replica_groups=[list(range(N_CORES))],
                    ins=[ag_in.opt()], outs=[ag_out.opt()])

                # -------- process gathered stats --------
                agsb = pa_w.tile([8, 2056], F32, name="agsb", bufs=1)
                nc.sync.dma_start(agsb[:], ag_out[:])
                rsfull = pa_w.tile([8, 2048], F32, name="rsfull", bufs=1)
                nc.gpsimd.partition_all_reduce(rsfull[:], agsb[:, 0:2048], 8,
                                               _rop("add"))
                g3 = pa_w.tile([8, 3], F32, name="g3", bufs=1)
                nc.gpsimd.partition_all_reduce(g3[:, 0:1], agsb[:, 2048:2049], 8,
                                               _rop("max"))
                nc.gpsimd.partition_all_reduce(g3[:, 1:2], agsb[:, 2049:2050], 8,
                                               _rop("max"))
                nc.gpsimd.partition_all_reduce(g3[:, 2:3], agsb[:, 2050:2051], 8,
                                               _rop("add"))
                nc.gpsimd.partition_broadcast(rsb[:], rsfull[0:1, 0:1024])
                ssqb = pa_w.tile([128, 1024], F32, name="ssqb", bufs=1)
                nc.gpsimd.partition_broadcast(ssqb[:], rsfull[0:1, 1024:2048])
                nc.gpsimd.partition_broadcast(scal[:, 2:3], g3[0:1, 0:1])  # -min
                nc.gpsimd.partition_broadcast(scal[:, 3:4], g3[0:1, 1:2])  # max
                nc.gpsimd.partition_broadcast(scal[:, 1:2], g3[0:1, 2:3])  # ssq
                nc.vector.reduce_sum(scal[:, 0:1], rsb[:], axis=AX)        # gsum

                # -------- scalar constants (same math as baseline) --------
                N_f = float(B) * float(D)
                nc.vector.tensor_single_scalar(scal[:, 4:5], scal[:, 0:1], 1.0 / N_f, A.mult)
                nc.vector.tensor_mul(scal[:, 20:21], scal[:, 0:1], scal[:, 4:5])
                nc.vector.tensor_sub(scal[:, 20:21], scal[:, 1:2], scal[:, 20:21])
                nc.vector.tensor_single_scalar(scal[:, 20:21], scal[:, 20:21],
                                               1.0 / (N_f - 1.0), A.mult)
                nc.scalar.activation(scal[:, 5:6], scal[:, 20:21], AF.Sqrt)
                nc.vector.reciprocal(scal[:, 6:7], scal[:, 5:6])
                nc.vector.tensor_single_scalar(scal[:, 21:22], scal[:, 2:3], -1.0, A.mult)
                nc.vector.tensor_sub(scal[:, 21:22], scal[:, 21:22], scal[:, 4:5])
                nc.vector.tensor_mul(scal[:, 7:8], scal[:, 21:22], scal[:, 6:7])
                nc.vector.tensor_sub(scal[:, 22:23], scal[:, 3:4], scal[:, 4:5])
                nc.vector.tensor_mul(scal[:, 22:23], scal[:, 22:23], scal[:, 6:7])
                nc.vector.tensor_sub(scal[:, 22:23], scal[:, 22:23], scal[:, 7:8])
                nc.vector.tensor_single_scalar(scal[:, 8:9], scal[:, 22:23],
                                               1.0 / BINS, A.mult)
                nc.vector.reciprocal(scal[:, 9:10], scal[:, 8:9])
                nc.vector.tensor_mul(scal[:, 10:11], scal[:, 6:7], scal[:, 9:10])
                nc.vector.tensor_mul(scal[:, 23:24], scal[:, 4:5], scal[:, 6:7])
                nc.vector.tensor_add(scal[:, 23:24], scal[:, 23:24], scal[:, 7:8])
                nc.vector.tensor_mul(scal[:, 23:24], scal[:, 23:24], scal[:, 9:10])
                nc.vector.tensor_single_scalar(scal[:, 11:12], scal[:, 23:24], -1.0, A.mult)
                nc.vector.tensor_single_scalar(scal[:, 16:17], scal[:, 8:9], float(D), A.mult)
                nc.vector.reciprocal(scal[:, 12:13], scal[:, 16:17])
                nc.vector.tensor_single_scalar(scal[:, 17:18], scal[:, 8:9], N_f, A.mult)
                nc.vector.reciprocal(scal[:, 13:14], scal[:, 17:18])
                nc.vector.tensor_single_scalar(scal[:, 14:15], scal[:, 12:13],
                                               -1.0 / LN2, A.mult)
                nc.vector.tensor_single_scalar(scal[:, 15:16], scal[:, 13:14],
                                               -1.0 / LN2, A.mult)

                # dmask[p, t*128+q] = sel[p, t] * (q == p); rs_own
                for t in range(8):
                    nc.vector.tensor_single_scalar(
                        dmask[:, t * 128:(t + 1) * 128], id128[:],
                        sel_sb[:, t:t + 1], A.mult)
                rtmp = pa_w.tile([128, 1024], F32, name="rtmp", bufs=1)
                nc.vector.tensor_mul(rtmp[:], rsb[:], dmask[:])
                nc.vector.reduce_sum(rs_own[:], rtmp[:], axis=AX)
                nc.vector.tensor_single_scalar(rs_own[:], rs_own[:],
                                               -1.0 / float(D), A.mult)
                # rdb = 1/d for ALL rows, locally: d^2 = ssqrow - rowsum^2/D
                nc.vector.tensor_mul(rtmp[:], rsb[:], rsb[:])
                nc.vector.scalar_tensor_tensor(rtmp[:], rtmp[:], -1.0 / float(D),
                                               ssqb[:], A.mult, A.add)
                nc.scalar.activation(rtmp[:], rtmp[:], AF.Sqrt)
                nc.vector.reciprocal(rdb[:], rtmp[:])
                rtm2 = pa_w.tile([128, 1024], F32, name="rtm2", bufs=1)
                nc.vector.tensor_mul(rtm2[:], rdb[:], dmask[:])
                nc.vector.reduce_sum(rd_own[:], rtm2[:], axis=AX)

                # GEMM: covp = xs @ xs.T (K-sharded partial), f32r.
                # Per-block staging on Act (phase B chunks 0..3 are Act-free)
                # with immediate per-block DMA into the cov_in bounce, so the
                # ReduceScatter input is ready right at GEMM end and phase A's
                # SBUF footprint stays small.
                cov_in_v = cov_in[:].rearrange("(m p) j -> p m j", p=128)
                for m in range(8):
                    for n2 in range(2):
                        pg = pa_ps2.tile([128, 512], F32, name="pg")
                        for k in range(16):
                            nc.tensor.matmul(
                                pg[:],
                                xst_r[:, k, m * 128:(m + 1) * 128],
                                xst_r[:, k, n2 * 512:(n2 + 1) * 512],
                                start=(k == 0), stop=(k == 15))
                        cvb = pa_w.tile([128, 512], F32, name="cvb", tag="cvb",
                                        bufs=4)
                        nc.scalar.copy(cvb[:], pg[:])
                        nc.sync.dma_start(
                            cov_in_v[:, m, n2 * 512:(n2 + 1) * 512], cvb[:])

            # ---------------- Phase B: histogram ----------------------------
            with (
                tc.tile_pool(name="hb_io", bufs=3) as hb_io,
                tc.tile_pool(name="hb_w", bufs=2) as hb_w,
                tc.tile_pool(name="hb_z", bufs=2) as hb_z,
                tc.tile_pool(name="hb_pl", bufs=2) as hb_pl,
                tc.tile_pool(name="hb_ps", bufs=6, space="PSUM") as hb_ps,
            ):
                pend_ent = []

                def _entropy_partial(er0):
                    lnc = hb_w.tile([32, C_ROWS * 32], F32, name="lnc",
                                    tag="ln", bufs=2)
                    nc.scalar.activation(
                        lnc[:], counts_sb[:, er0 * 32:(er0 + C_ROWS) * 32],
                        AF.Ln, scale=scal[0:32, 12:13], bias=epsb[0:32, :])
                    nc.vector.tensor_mul(
                        lnc[:], lnc[:], counts_sb[:, er0 * 32:(er0 + C_ROWS) * 32])
                    nc.vector.reduce_sum(
                        erp_all[:, er0:er0 + C_ROWS],
                        lnc[:].rearrange("p (r l) -> p r l", r=C_ROWS), axis=AX)

                for ch in range(N_CH):
                    act_free = ch < ACT_FREE_CH
                    r0 = ch * C_ROWS
                    xch = hb_io.tile([128, C_ROWS, 128], F32, name="xch")
                    nc.sync.dma_start(xch[:], xr_v[:, r0:r0 + C_ROWS, :])
                    xf = xch[:].rearrange("p a b -> p (a b)")

                    # v = SC*x + BC (Act: +Relu; DVE fallback lets the rare
                    # v<0 element drop from the histogram, which is harmless)
                    v = hb_w.tile([128, F], F32, name="v", tag="ew", bufs=EW_BUFS)
                    if act_free:
                        nc.vector.tensor_scalar(v[:], xf, scal[:, 10:11],
                                                scal[:, 11:12], A.mult, A.add)
                    else:
                        nc.scalar.activation(v[:], xf, AF.Relu,
                                             bias=scal[:, 11:12],
                                             scale=scal[:, 10:11])
                    i1 = hb_w.tile([128, F], I32, name="i1", tag="ew", bufs=EW_BUFS)
                    nc.vector.tensor_single_scalar(i1[:], v[:], 999.49, A.min)
                    g1 = hb_w.tile([128, F], F32, name="g1", tag="ew", bufs=EW_BUFS)
                    nc.vector.tensor_tensor(g1[:], i1[:], v[:], A.is_gt)
                    idxf = hb_w.tile([128, F], F32, name="idxf", tag="ew", bufs=EW_BUFS)
                    nc.vector.tensor_tensor(idxf[:], i1[:], g1[:], A.subtract)
                    ihi = hb_w.tile([128, F], I32, name="ihi", tag="ew", bufs=EW_BUFS)
                    nc.vector.tensor_scalar(ihi[:], idxf[:], 1.0 / 32.0, -0.484375,
                                            A.mult, A.add)
                    z2 = hb_z.tile([128, 4 * F], BF16, name="z2")
                    nc.vector.tensor_copy(z2[:, 0:F], ihi[:])
                    nc.vector.scalar_tensor_tensor(z2[:, F:2 * F], ihi[:], -32.0,
                                                   idxf[:], A.mult, A.add)
                    nc.vector.tensor_single_scalar(z2[:, 2 * F:4 * F],
                                                   z2[:, 0:2 * F], 16.0,
                                                   A.subtract)

                    # pair planes: one is_equal over z2=[z | z-16] yields planes
                    # (j, j+16) stored interleaved at slots (2j, 2j+1).  All
                    # downstream uses of counts are permutation-invariant.
                    P = hb_pl.tile([128, 32, 2 * F], BF16, name="P")
                    Pv = P[:].rearrange("p q f -> p (q f)")
                    n_dve = PR_DVE + (1 if act_free else 0)
                    n_gps = PR_GPS + (1 if act_free else 0)
                    for j in range(n_dve):
                        nc.vector.tensor_single_scalar(
                            Pv[:, j * 4 * F:(j + 1) * 4 * F], z2[:], float(j),
                            A.is_equal)
                    for j in range(n_dve, n_dve + n_gps):
                        nc.gpsimd.tensor_single_scalar(
                            Pv[:, j * 4 * F:(j + 1) * 4 * F], z2[:], float(j),
                            A.is_equal)
                    for j in range(n_dve + n_gps, 16):
                        atmp = hb_w.tile([128, 4 * F], BF16, name="atmp",
                                         tag="at", bufs=1)
                        nc.scalar.activation(atmp[:], z2[:], AF.Square,
                                             bias=negq[:, j:j + 1])
                        nc.scalar.activation(Pv[:, j * 4 * F:(j + 1) * 4 * F],
                                             atmp[:], AF.Relu,
                                             bias=one1[:], scale=-1.0)

                    for r in range(C_ROWS):
                        ps = hb_ps.tile([32, 32], F32, name="ps")
                        for k in range(128):
                            t = r * 128 + k
                            nc.tensor.matmul(ps[:], P[:, :, t], P[:, :, F + t],
                                             start=(k == 0), stop=(k == 127))
                        rr = r0 + r
                        if act_free:
                            nc.vector.tensor_copy(
                                counts_sb[:, rr * 32:(rr + 1) * 32], ps[:32, :])
                        else:
                            nc.scalar.copy(
                                counts_sb[:, rr * 32:(rr + 1) * 32], ps[:32, :])

                    # per-chunk local-entropy partials (deferred while Act-free)
                    if act_free:
                        pend_ent.append(r0)
                    else:
                        for pr0 in pend_ent:
                            _entropy_partial(pr0)
                        pend_ent = []
                        _entropy_partial(r0)

                    # hidden mid-phase work
                    if ch == 5:
                        nc.gpsimd.collective_compute(
                            "ReduceScatter", A.add,
                            replica_groups=[list(range(N_CORES))],
                            ins=[cov_in.opt()], outs=[cov_out.opt()])
                    if ch == 13:
                        nc.sync.dma_start(covc[:], cov_out[:])
                        nc.vector.scalar_tensor_tensor(covc[:], rsb[:], rs_own[:],
                                                       covc[:], A.mult, A.add)
                    if ch == 15:
                        t1 = hb_w.tile([128, 1024], F32, name="t1", tag="dt", bufs=2)
                        nc.vector.tensor_mul(t1[:], covc[:], rdb[:])
                        nc.vector.tensor_single_scalar(t1[:], t1[:], rd_own[:],
                                                       A.mult)
                        t1c = hb_w.tile([128, 1024], F32, name="t1c", tag="dt", bufs=2)
                        nc.scalar.activation(t1c[:], t1[:], AF.Abs,
                                             accum_out=f1own[:])
                        nc.vector.tensor_single_scalar(f1own[:], f1own[:],
                                                       1.0 / float(B), A.mult)

            # ---------------- Phase C: entropies + mask ----------------------
            with (
                tc.tile_pool(name="pc_w", bufs=2) as pc_w,
                tc.tile_pool(name="pc_ps", bufs=2, space="PSUM") as pc_ps,
                tc.tile_pool(name="pc_io", bufs=6) as pc_io,
            ):
                # prefetch mask-phase loads (independent of pvec)
                xm_t = []
                um_t = []
                CH = 2048
                for c in range(D // CH):
                    xm = pc_io.tile([128, CH], F32, name="xm", tag="xm", bufs=3)
                    um = pc_io.tile([128, CH], F32, name="um", tag="um", bufs=3)
                    nc.sync.dma_start(xm[:], xr.ap()[:, c * CH:(c + 1) * CH])
                    nc.sync.dma_start(um[:], ur.ap()[:, c * CH:(c + 1) * CH])
                    xm_t.append(xm)
                    um_t.append(um)

                # batch-count partial + exchange
                bc_part = pc_w.tile([32, 32], F32, name="bc_part", bufs=1)
                nc.vector.reduce_sum(
                    bc_part[:],
                    counts_sb[:].rearrange("p (r l) -> p l r", r=RSL), axis=AX)
                nc.sync.dma_start(bc_in[:], bc_part[:])
                nc.gpsimd.collective_compute(
                    "AllGather", A.bypass,
                    replica_groups=[list(range(N_CORES))],
                    ins=[bc_in.opt()], outs=[bc_out.opt()])

                # local entropies from accumulated partials (hidden under AG)
                psS = pc_ps.tile([1, RSL], F32, name="psS")
                nc.tensor.matmul(psS[:], ones32[:], erp_all[:], start=True,
                                 stop=True)
                srow = pc_w.tile([1, RSL], F32, name="srow", bufs=1)
                nc.scalar.copy(srow[:], psS[:])
                psT = pc_ps.tile([128, 1], F32, name="psT")
                nc.tensor.transpose(psT[:], srow[:], id128[:1, :1])
                hloc = pc_w.tile([128, 1], F32, name="hloc", bufs=1)
                nc.scalar.copy(hloc[:], psT[:])
                nc.vector.tensor_mul(hloc[:], hloc[:], scal[:, 14:15])

                agb = pc_w.tile([32, 8, 32], F32, name="agb", bufs=1)
                nc.sync.dma_start(
                    agb[:], bc_out[:].rearrange("(r p) c -> p r c", p=32))
                bcs = pc_w.tile([32, 32], F32, name="bcs", bufs=1)
                nc.vector.reduce_sum(
                    bcs[:], agb[:].rearrange("p r c -> p c r"), axis=AX)
                lnb = pc_w.tile([32, 32], F32, name="lnb", bufs=1)
                nc.scalar.activation(lnb[:], bcs[:], AF.Ln,
                                     scale=scal[0:32, 13:14], bias=epsb[0:32, :])
                nc.vector.tensor_mul(lnb[:], lnb[:], bcs[:])
                sb1 = pc_w.tile([32, 1], F32, name="sb1", bufs=1)
                nc.vector.reduce_sum(sb1[:], lnb[:], axis=AX)
                nc.gpsimd.partition_all_reduce(sb1[:], sb1[:], 32, _rop("add"))
                hbat = pc_w.tile([128, 1], F32, name="hbat", bufs=1)
                nc.gpsimd.partition_broadcast(hbat[:], sb1[0:1, :])
                nc.vector.tensor_mul(hbat[:], hbat[:], scal[:, 15:16])

                # f2' = max(f2, 1/f2); keep = f1/f2'; p = 1-keep; rkeep = 1/keep
                tA = pc_w.tile([128, 1], F32, name="tA", bufs=1)
                tB = pc_w.tile([128, 1], F32, name="tB", bufs=1)
                nc.vector.reciprocal(tA[:], hbat[:])
                f2 = pc_w.tile([128, 1], F32, name="f2", bufs=1)
                nc.vector.tensor_mul(f2[:], hloc[:], tA[:])
                nc.vector.reciprocal(tB[:], f2[:])
                nc.vector.tensor_max(f2[:], f2[:], tB[:])
                nc.vector.reciprocal(tB[:], f2[:])
                nc.vector.tensor_mul(pvec[:, 2:3], f1own[:], tB[:])
                nc.vector.tensor_scalar(pvec[:, 0:1], pvec[:, 2:3], -1.0, 1.0,
                                        A.mult, A.add)
                nc.vector.reciprocal(pvec[:, 1:2], pvec[:, 2:3])

                if DEBUG:
                    dbg_sb = pc_w.tile([128, 16], F32, name="dbg_sb", bufs=1)
                    nc.scalar.copy(dbg_sb[:, 0:4], pvec[:])
                    nc.scalar.copy(dbg_sb[:, 4:5], hloc[:])
                    nc.scalar.copy(dbg_sb[:, 5:6], hbat[:])
                    nc.scalar.copy(dbg_sb[:, 6:14], scal[:, 4:12])
                    nc.scalar.copy(dbg_sb[:, 14:15], f1own[:])
                    nc.scalar.copy(dbg_sb[:, 15:16], rd_own[:])
                    nc.sync.dma_start(dbg.ap(), dbg_sb[:])

                # mask + scale
                for c in range(D // CH):
                    xm, um = xm_t[c], um_t[c]
                    nc.vector.tensor_single_scalar(um[:], um[:], pvec[:, 0:1],
                                                   A.is_gt)
                    oc = pc_io.tile([128, CH], F32, name="oc", tag="oc", bufs=3)
                    nc.vector.scalar_tensor_tensor(oc[:], um[:], pvec[:, 1:2],
                                                   xm[:], A.mult, A.mult)
                    nc.sync.dma_start(out.ap()[:, c * CH:(c + 1) * CH], oc[:])

    nc.compile()
    return nc


def _rop(name):
    from concourse import bass_isa
    return getattr(bass_isa.ReduceOp, name)


def _bcast_cols(nc, sbuf_pool, psum_pool, vec8, dst, id128):
    """dst[p, t*128+q] = vec8[q, t]  (flatten [128,8] col-major, bcast to all
    partitions)."""
    import concourse.mybir as mybir
    F32 = mybir.dt.float32
    pt = psum_pool.tile([8, 128], F32, name="bc_pt")
    nc.tensor.transpose(pt[:8, :], vec8[:], id128[:])
    tr = sbuf_pool.tile([8, 128], F32, name="bc_tr", bufs=1)
    nc.scalar.copy(tr[:], pt[:8, :])
    flat = sbuf_pool.tile([1, 8 * 128], F32, name="bc_flat", bufs=1)
    for t in range(8):
        nc.sync.dma_start(flat[:, t * 128:(t + 1) * 128], tr[t:t + 1, :])
    nc.gpsimd.partition_broadcast(dst[:], flat[:])


def kernel(x, u):
    if "nc" not in _cache:
        _cache["nc"] = _build()
    nc = _cache["nc"]
    from concourse.bass_utils import run_bass_kernel_spmd

    x = np.asarray(x, dtype=np.float32)
    u = np.asarray(u, dtype=np.float32)
    orig_shape = x.shape
    xf = np.ascontiguousarray(x.reshape(B, D))
    uf = np.ascontiguousarray(u.reshape(B, D))
    in_maps = []
    for c in range(N_CORES):
        selv = np.zeros((128, 8), np.float32)
        selv[:, c] = 1.0
        in_maps.append({
            "xst": np.ascontiguousarray(xf[:, c * DSL:(c + 1) * DSL].T),
            "xr": np.ascontiguousarray(xf[c * RSL:(c + 1) * RSL, :]),
            "ur": np.ascontiguousarray(uf[c * RSL:(c + 1) * RSL, :]),
            "sel": selv,
        })
    res = run_bass_kernel_spmd(nc, in_maps, core_ids=list(range(N_CORES)))
    _cache["last_results"] = res
    outf = np.concatenate([res.results[c]["out"] for c in range(N_CORES)], axis=0)
    return outf.reshape(orig_shape)


# revision 14
# speedup vs baseline: 1.4333x; 1.0317x over previous
"""DifferentialDropout Trainium2 kernel (8-core SPMD), v3.

Reference semantics: per-row corrcoef factor f1, global-standardized 1000-bin
per-row histograms -> entropies -> per-row keep prob -> mask+scale.

Key design (vs 771.8 us baseline):
  - cov GEMM in float32r (1 cycle/row vs fp32's 4) with explicit rounding
    copies; rowsums via PE matmul on the same f32r tiles.
  - phase A stats computed from the xst (D-shard) tiles while they stream;
    ONE combined AllGather carries rowsum partials + min/max/sumsq, issued
    before the GEMM so phase B starts ~50 us.
  - histogram prep: fused floor chain (7 DVE ops/chunk) writing
    z2 = [hi | lo | hi-16 | lo-16]; ONE is_equal over z2 yields TWO planes
    (j, j+16) stored pair-interleaved (entropy is permutation-invariant
    over bins, so the scramble never needs undoing).
  - plane pairs split DVE/Act/Pool; Act uses Square+Relu pairs.
  - chunks 0-3 are Act-free (v on DVE, pairs redistributed, entropies
    deferred) so the GEMM's PSUM->SBUF staging copies can run on Act
    without head-blocking phase B; cov ReduceScatter issued at chunk 5,
    d AllGather at chunk 13 - all hidden under phase B.
  - mask-phase x/u loads prefetched before the batch-count exchange.
"""

import os
import sys

sys.path.insert(0, "/opt/trn_rl_repo")

import numpy as np

B = 1024
D = 16384
BINS = 1000
N_CORES = 8
DSL = D // N_CORES      # 2048
RSL = B // N_CORES      # 128
C_ROWS = 4
F = C_ROWS * 128        # 512 elems/partition/chunk
N_CH = RSL // C_ROWS    # 32 chunks
LN2 = 0.6931471805599453

# plane split: 16 pair-planes per chunk across engines
PR_DVE = int(os.environ.get("K_PRDVE", "11"))
PR_ACT = int(os.environ.get("K_PRACT", "2"))
PR_GPS = 16 - PR_DVE - PR_ACT
ACT_FREE_CH = int(os.environ.get("K_AFCH", "4"))   # Act-free leading chunks
EW_BUFS = int(os.environ.get("K_EWB", "6"))
DEBUG = os.environ.get("K_DEBUG", "0") == "1"

_cache = {}


def _build():
    import concourse.mybir as mybir
    import concourse.tile as tile
    from concourse import bacc
    from concourse.masks import make_identity

    F32 = mybir.dt.float32
    F32R = mybir.dt.float32r
    BF16 = mybir.dt.bfloat16
    I32 = mybir.dt.int32
    A = mybir.AluOpType
    AF = mybir.ActivationFunctionType
    AX = mybir.AxisListType.X

    nc = bacc.Bacc("TRN2", target_bir_lowering=False, debug=False,
                   num_devices=N_CORES)

    xst = nc.dram_tensor("xst", [DSL, B], F32, kind="ExternalInput")
    xr = nc.dram_tensor("xr", [RSL, D], F32, kind="ExternalInput")
    ur = nc.dram_tensor("ur", [RSL, D], F32, kind="ExternalInput")
    sel = nc.dram_tensor("sel", [128, 8], F32, kind="ExternalInput")
    out = nc.dram_tensor("out", [RSL, D], F32, kind="ExternalOutput")
    if DEBUG:
        dbg = nc.dram_tensor("dbg", [128, 16], F32, kind="ExternalOutput")

    xr_v = xr.ap().rearrange("r (p e) -> p r e", p=128)   # [128, 128, 128]

    with tile.TileContext(nc) as tc:
        with (
            tc.tile_pool(name="const", bufs=1) as constp,
            tc.tile_pool(name="persist", bufs=1) as persist,
            tc.tile_pool(name="dram", bufs=1, space="DRAM") as dram,
        ):
            id128 = constp.tile([128, 128], F32, name="id128")
            make_identity(nc, id128[:])
            ones32 = constp.tile([32, 1], F32, name="ones32")
            nc.vector.memset(ones32[:], 1.0)
            ones1f = constp.tile([128, 1], F32, name="ones1f")
            nc.vector.memset(ones1f[:], 1.0)
            ones1r = constp.tile([128, 1], F32R, name="ones1r")
            nc.vector.tensor_copy(ones1r[:], ones1f[:])
            epsb = constp.tile([128, 1], F32, name="epsb")
            nc.vector.memset(epsb[:], 1e-30)
            one1 = constp.tile([128, 1], F32, name="one1")
            nc.vector.memset(one1[:], 1.0)
            negq = constp.tile([128, 16], F32, name="negq")
            for _q in range(16):
                nc.vector.memset(negq[:, _q:_q + 1], -float(_q))

            # persistent SBUF
            counts_sb = persist.tile([32, RSL * 32], F32, name="counts_sb")
            erp_all = persist.tile([32, RSL], F32, name="erp_all")
            scal = persist.tile([128, 24], F32, name="scal")
            rsb = persist.tile([128, 1024], F32, name="rsb")
            rdb = persist.tile([128, 1024], F32, name="rdb")
            dmask = persist.tile([128, 1024], F32, name="dmask")
            covc = persist.tile([128, 1024], F32, name="covc")
            sel_sb = persist.tile([128, 8], F32, name="sel_sb")
            rs_own = persist.tile([128, 1], F32, name="rs_own")
            rd_own = persist.tile([128, 1], F32, name="rd_own")
            f1own = persist.tile([128, 1], F32, name="f1own")
            pvec = persist.tile([128, 4], F32, name="pvec")

            nc.sync.dma_start(sel_sb[:], sel.ap())

            # DRAM bounces
            ag_in = dram.tile([1, 2056], F32, name="ag_in")
            ag_out = dram.tile([8, 2056], F32, addr_space="Shared", name="ag_out")
            cov_in = dram.tile([1024, 1024], F32, name="cov_in")
            cov_out = dram.tile([128, 1024], F32, name="cov_out")
            bc_in = dram.tile([32, 32], F32, name="bc_in")
            bc_out = dram.tile([256, 32], F32, addr_space="Shared", name="bc_out")

            # ---------------- Phase A: stats + GEMM (f32r) -------------------
            with (
                tc.tile_pool(name="pa_io", bufs=3) as pa_io,
                tc.tile_pool(name="pa_big", bufs=1) as pa_big,
                tc.tile_pool(name="pa_w", bufs=2) as pa_w,
                tc.tile_pool(name="pa_ps2", bufs=2, space="PSUM") as pa_ps2,
                tc.tile_pool(name="pa_rs", bufs=1, space="PSUM") as pa_rs,
            ):
                xst_r = pa_big.tile([128, 16, 1024], F32R, name="xst_r")
                mn16 = pa_big.tile([128, 16], F32, name="mn16")
                mx16 = pa_big.tile([128, 16], F32, name="mx16")
                ssq16 = pa_big.tile([128, 16], F32, name="ssq16")
                rs_ps = pa_rs.tile([1, 2, 512], F32, name="rs_ps")
                rs2_ps = pa_rs.tile([1, 2, 512], F32, name="rs2_ps")
                for k in range(16):
                    xk = pa_io.tile([128, 1024], F32, name="xk")
                    nc.sync.dma_start(xk[:], xst.ap()[k * 128:(k + 1) * 128, :])
                    # f32r rounding copy on Pool; min+max reduces on DVE
                    nc.gpsimd.tensor_copy(xst_r[:, k, :], xk[:])
                    nc.vector.reduce_sum(mn16[:, k:k + 1], xk[:], axis=AX, op=A.min)
                    nc.vector.reduce_sum(mx16[:, k:k + 1], xk[:], axis=AX, op=A.max)
                    sq = pa_w.tile([128, 1024], F32R, name="sq", tag="sq", bufs=2)
                    nc.scalar.activation(sq[:], xk[:], AF.Square,
                                         accum_out=ssq16[:, k:k + 1])
                    # rowsum + row-sumsq partials via PE (accumulate over k)
                    nc.tensor.matmul(rs_ps[:, 0, :], ones1r[:], xst_r[:, k, 0:512],
                                     start=(k == 0), stop=(k == 15))
                    nc.tensor.matmul(rs_ps[:, 1, :], ones1r[:], xst_r[:, k, 512:1024],
                                     start=(k == 0), stop=(k == 15))
                    nc.tensor.matmul(rs2_ps[:, 0, :], ones1r[:], sq[:, 0:512],
                                     start=(k == 0), stop=(k == 15))
                    nc.tensor.matmul(rs2_ps[:, 1, :], ones1r[:], sq[:, 512:1024],
                                     start=(k == 0), stop=(k == 15))

                # local reductions -> per-partition partials
                pmn = pa_w.tile([128, 1], F32, name="pmn", bufs=1)
                nc.vector.reduce_sum(pmn[:], mn16[:], axis=AX, op=A.min)
                nc.vector.tensor_single_scalar(pmn[:], pmn[:], -1.0, A.mult)
                pmx = pa_w.tile([128, 1], F32, name="pmx", bufs=1)
                nc.vector.reduce_sum(pmx[:], mx16[:], axis=AX, op=A.max)
                pss = pa_w.tile([128, 1], F32, name="pss", bufs=1)
                nc.vector.reduce_sum(pss[:], ssq16[:], axis=AX)
                # cross-partition (128) reduce
                nc.gpsimd.partition_all_reduce(pmn[:], pmn[:], 128, _rop("max"))
                nc.gpsimd.partition_all_reduce(pmx[:], pmx[:], 128, _rop("max"))
                nc.gpsimd.partition_all_reduce(pss[:], pss[:], 128, _rop("add"))
                rs_sb = pa_w.tile([1, 2048], F32, name="rs_sb", bufs=1)
                nc.scalar.copy(rs_sb[:, 0:512], rs_ps[:, 0, :])
                nc.scalar.copy(rs_sb[:, 512:1024], rs_ps[:, 1, :])
                nc.scalar.copy(rs_sb[:, 1024:1536], rs2_ps[:, 0, :])
                nc.scalar.copy(rs_sb[:, 1536:2048], rs2_ps[:, 1, :])
                nc.sync.dma_start(ag_in[:, 0:2048], rs_sb[:])
                nc.sync.dma_start(ag_in[:, 2048:2049], pmn[0:1, :])
                nc.sync.dma_start(ag_in[:, 2049:2050], pmx[0:1, :])
                nc.sync.dma_start(ag_in[:, 2050:2051], pss[0:1, :])
                nc.gpsimd.collective_compute(
                    "AllGather", A.bypass,
                    replica_groups=[list(range(N_CORES))],
                    ins=[ag_in.opt()], outs=[ag_out.opt()])

                # -------- process gathered stats --------
                agsb = pa_w.tile([8, 2056], F32, name="agsb", bufs=1)
                nc.sync.dma_start(agsb[:], ag_out[:])
                rsfull = pa_w.tile([8, 2048], F32, name="rsfull", bufs=1)
                nc.gpsimd.partition_all_reduce(rsfull[:], agsb[:, 0:2048], 8,
                                               _rop("add"))
                g3 = pa_w.tile([8, 3], F32, name="g3", bufs=1)
                nc.gpsimd.partition_all_reduce(g3[:, 0:1], agsb[:, 2048:2049], 8,
                                               _rop("max"))
                nc.gpsimd.partition_all_reduce(g3[:, 1:2], agsb[:, 2049:2050], 8,
                                               _rop("max"))
                nc.gpsimd.partition_all_reduce(g3[:, 2:3], agsb[:, 2050:2051], 8,
                                               _rop("add"))
                nc.gpsimd.partition_broadcast(rsb[:], rsfull[0:1, 0:1024])
                ssqb = pa_w.tile([128, 1024], F32, name="ssqb", bufs=1)
                nc.gpsimd.partition_broadcast(ssqb[:], rsfull[0:1, 1024:2048])
                nc.gpsimd.partition_broadcast(scal[:, 2:3], g3[0:1, 0:1])  # -min
                nc.gpsimd.partition_broadcast(scal[:, 3:4], g3[0:1, 1:2])  # max
                nc.gpsimd.partition_broadcast(scal[:, 1:2], g3[0:1, 2:3])  # ssq
                nc.vector.reduce_sum(scal[:, 0:1], rsb[:], axis=AX)        # gsum

                # -------- scalar constants (same math as baseline) --------
                N_f = float(B) * float(D)
                nc.vector.tensor_single_scalar(scal[:, 4:5], scal[:, 0:1], 1.0 / N_f, A.mult)
                nc.vector.tensor_mul(scal[:, 20:21], scal[:, 0:1], scal[:, 4:5])
                nc.vector.tensor_sub(scal[:, 20:21], scal[:, 1:2], scal[:, 20:21])
                nc.vector.tensor_single_scalar(scal[:, 20:21], scal[:, 20:21],
                                               1.0 / (N_f - 1.0), A.mult)
                nc.scalar.activation(scal[:, 5:6], scal[:, 20:21], AF.Sqrt)
                nc.vector.reciprocal(scal[:, 6:7], scal[:, 5:6])
                nc.vector.tensor_single_scalar(scal[:, 21:22], scal[:, 2:3], -1.0, A.mult)
                nc.vector.tensor_sub(scal[:, 21:22], scal[:, 21:22], scal[:, 4:5])
                nc.vector.tensor_mul(scal[:, 7:8], scal[:, 21:22], scal[:, 6:7])
                nc.vector.tensor_sub(scal[:, 22:23], scal[:, 3:4], scal[:, 4:5])
                nc.vector.tensor_mul(scal[:, 22:23], scal[:, 22:23], scal[:, 6:7])
                nc.vector.tensor_sub(scal[:, 22:23], scal[:, 22:23], scal[:, 7:8])
                nc.vector.tensor_single_scalar(scal[:, 8:9], scal[:, 22:23],
                                               1.0 / BINS, A.mult)
                nc.vector.reciprocal(scal[:, 9:10], scal[:, 8:9])
                nc.vector.tensor_mul(scal[:, 10:11], scal[:, 6:7], scal[:, 9:10])
                nc.vector.tensor_mul(scal[:, 23:24], scal[:, 4:5], scal[:, 6:7])
                nc.vector.tensor_add(scal[:, 23:24], scal[:, 23:24], scal[:, 7:8])
                nc.vector.tensor_mul(scal[:, 23:24], scal[:, 23:24], scal[:, 9:10])
                nc.vector.tensor_single_scalar(scal[:, 11:12], scal[:, 23:24], -1.0, A.mult)
                nc.vector.tensor_single_scalar(scal[:, 16:17], scal[:, 8:9], float(D), A.mult)
                nc.vector.reciprocal(scal[:, 12:13], scal[:, 16:17])
                nc.vector.tensor_single_scalar(scal[:, 17:18], scal[:, 8:9], N_f, A.mult)
                nc.vector.reciprocal(scal[:, 13:14], scal[:, 17:18])
                nc.vector.tensor_single_scalar(scal[:, 14:15], scal[:, 12:13],
                                               -1.0 / LN2, A.mult)
                nc.vector.tensor_single_scalar(scal[:, 15:16], scal[:, 13:14],
                                               -1.0 / LN2, A.mult)

                # dmask[p, t*128+q] = sel[p, t] * (q == p); rs_own
                for t in range(8):
                    nc.vector.tensor_single_scalar(
                        dmask[:, t * 128:(t + 1) * 128], id128[:],
                        sel_sb[:, t:t + 1], A.mult)
                rtmp = pa_w.tile([128, 1024], F32, name="rtmp", bufs=1)
                nc.vector.tensor_mul(rtmp[:], rsb[:], dmask[:])
                nc.vector.reduce_sum(rs_own[:], rtmp[:], axis=AX)
                nc.vector.tensor_single_scalar(rs_own[:], rs_own[:],
                                               -1.0 / float(D), A.mult)
                # rdb = 1/d for ALL rows, locally: d^2 = ssqrow - rowsum^2/D
                nc.vector.tensor_mul(rtmp[:], rsb[:], rsb[:])
                nc.vector.scalar_tensor_tensor(rtmp[:], rtmp[:], -1.0 / float(D),
                                               ssqb[:], A.mult, A.add)
                nc.scalar.activation(rtmp[:], rtmp[:], AF.Sqrt)
                nc.vector.reciprocal(rdb[:], rtmp[:])
                rtm2 = pa_w.tile([128, 1024], F32, name="rtm2", bufs=1)
                nc.vector.tensor_mul(rtm2[:], rdb[:], dmask[:])
                nc.vector.reduce_sum(rd_own[:], rtm2[:], axis=AX)

                # GEMM: covp = xs @ xs.T (K-sharded partial), f32r.
                # Per-block staging on Act (phase B chunks 0..3 are Act-free)
                # with immediate per-block DMA into the cov_in bounce, so the
                # ReduceScatter input is ready right at GEMM end and phase A's
                # SBUF footprint stays small.
                cov_in_v = cov_in[:].rearrange("(m p) j -> p m j", p=128)
                for m in range(8):
                    for n2 in range(2):
                        pg = pa_ps2.tile([128, 512], F32, name="pg")
                        for k in range(16):
                            nc.tensor.matmul(
                                pg[:],
                                xst_r[:, k, m * 128:(m + 1) * 128],
                                xst_r[:, k, n2 * 512:(n2 + 1) * 512],
                                start=(k == 0), stop=(k == 15))
                        cvb = pa_w.tile([128, 512], F32, name="cvb", tag="cvb",
                                        bufs=4)
                        nc.scalar.copy(cvb[:], pg[:])
                        nc.sync.dma_start(
                            cov_in_v[:, m, n2 * 512:(n2 + 1) * 512], cvb[:])

            # ---------------- Phase B: histogram ----------------------------
            with (
                tc.tile_pool(name="hb_io", bufs=3) as hb_io,
                tc.tile_pool(name="hb_w", bufs=2) as hb_w,
                tc.tile_pool(name="hb_z", bufs=2) as hb_z,
                tc.tile_pool(name="hb_pl", bufs=2) as hb_pl,
                tc.tile_pool(name="hb_ps", bufs=6, space="PSUM") as hb_ps,
            ):
                pend_ent = []

                def _entropy_partial(er0):
                    lnc = hb_w.tile([32, C_ROWS * 32], F32, name="lnc",
                                    tag="ln", bufs=2)
                    nc.scalar.activation(
                        lnc[:], counts_sb[:, er0 * 32:(er0 + C_ROWS) * 32],
                        AF.Ln, scale=scal[0:32, 12:13], bias=epsb[0:32, :])
                    nc.vector.tensor_mul(
                        lnc[:], lnc[:], counts_sb[:, er0 * 32:(er0 + C_ROWS) * 32])
                    nc.vector.reduce_sum(
                        erp_all[:, er0:er0 + C_ROWS],
                        lnc[:].rearrange("p (r l) -> p r l", r=C_ROWS), axis=AX)

                for ch in range(N_CH):
                    act_free = ch < ACT_FREE_CH
                    r0 = ch * C_ROWS
                    xch = hb_io.tile([128, C_ROWS, 128], F32, name="xch")
                    nc.sync.dma_start(xch[:], xr_v[:, r0:r0 + C_ROWS, :])
                    xf = xch[:].rearrange("p a b -> p (a b)")

                    # v = SC*x + BC (Act: +Relu; DVE fallback lets the rare
                    # v<0 element drop from the histogram, which is harmless)
                    v = hb_w.tile([128, F], F32, name="v", tag="ew", bufs=EW_BUFS)
                    if act_free:
                        nc.vector.tensor_scalar(v[:], xf, scal[:, 10:11],
                                                scal[:, 11:12], A.mult, A.add)
                    else:
                        nc.scalar.activation(v[:], xf, AF.Relu,
                                             bias=scal[:, 11:12],
                                             scale=scal[:, 10:11])
                    i1 = hb_w.tile([128, F], I32, name="i1", tag="ew", bufs=EW_BUFS)
                    nc.vector.tensor_single_scalar(i1[:], v[:], 999.49, A.min)
                    g1 = hb_w.tile([128, F], F32, name="g1", tag="ew", bufs=EW_BUFS)
                    nc.vector.tensor_tensor(g1[:], i1[:], v[:], A.is_gt)
                    idxf = hb_w.tile([128, F], F32, name="idxf", tag="ew", bufs=EW_BUFS)
                    nc.vector.tensor_tensor(idxf[:], i1[:], g1[:], A.subtract)
                    ihi = hb_w.tile([128, F], I32, name="ihi", tag="ew", bufs=EW_BUFS)
                    nc.vector.tensor_scalar(ihi[:], idxf[:], 1.0 / 32.0, -0.484375,
                                            A.mult, A.add)
                    z2 = hb_z.tile([128, 4 * F], BF16, name="z2")
                    nc.vector.tensor_copy(z2[:, 0:F], ihi[:])
                    nc.vector.scalar_tensor_tensor(z2[:, F:2 * F], ihi[:], -32.0,
                                                   idxf[:], A.mult, A.add)
                    nc.vector.tensor_single_scalar(z2[:, 2 * F:4 * F],
                                                   z2[:, 0:2 * F], 16.0,
                                                   A.subtract)

                    # pair planes: one is_equal over z2=[z | z-16] yields planes
                    # (j, j+16) stored interleaved at slots (2j, 2j+1).  All
                    # downstream uses of counts are permutation-invariant.
                    P = hb_pl.tile([128, 32, 2 * F], BF16, name="P")
                    Pv = P[:].rearrange("p q f -> p (q f)")
                    n_dve = PR_DVE + (1 if act_free else 0)
                    n_gps = PR_GPS + (1 if act_free else 0)
                    for j in range(n_dve):
                        nc.vector.tensor_single_scalar(
                            Pv[:, j * 4 * F:(j + 1) * 4 * F], z2[:], float(j),
                            A.is_equal)
                    for j in range(n_dve, n_dve + n_gps):
                        nc.gpsimd.tensor_single_scalar(
                            Pv[:, j * 4 * F:(j + 1) * 4 * F], z2[:], float(j),
                            A.is_equal)
                    for j in range(n_dve + n_gps, 16):
                        atmp = hb_w.tile([128, 4 * F], BF16, name="atmp",
                                         tag="at", bufs=1)
                        nc.scalar.activation(atmp[:], z2[:], AF.Square,
                                             bias=negq[:, j:j + 1])
                        nc.scalar.activation(Pv[:, j * 4 * F:(j + 1) * 4 * F],
                                             atmp[:], AF.Relu,
                                             bias=one1[:], scale=-1.0)

                    for r in range(C_ROWS):
                        ps = hb_ps.tile([32, 32], F32, name="ps")
                        for k in range(128):
                            t = r * 128 + k
                            nc.tensor.matmul(ps[:], P[:, :, t], P[:, :, F + t],
                                             start=(k == 0), stop=(k == 127))
                        rr = r0 + r
                        if act_free:
                            nc.vector.tensor_copy(
                                counts_sb[:, rr * 32:(rr + 1) * 32], ps[:32, :])
                        else:
                            nc.scalar.copy(
                                counts_sb[:, rr * 32:(rr + 1) * 32], ps[:32, :])

                    # per-chunk local-entropy partials (deferred while Act-free)
                    if act_free:
                        pend_ent.append(r0)
                    else:
                        for pr0 in pend_ent:
                            _entropy_partial(pr0)
                        pend_ent = []
                        _entropy_partial(r0)

                    # hidden mid-phase work
                    if ch == 5:
                        nc.gpsimd.collective_compute(
                            "ReduceScatter", A.add,
                            replica_groups=[list(range(N_CORES))],
                            ins=[cov_in.opt()], outs=[cov_out.opt()])
                    if ch == 13:
                        # pinned late so the scheduler cannot hoist these into
                        # early engine-queue positions (in-order engines would
                        # head-block on the ReduceScatter result otherwise)
                        with tc.tile_wait_until(0.26):
                            nc.sync.dma_start(covc[:], cov_out[:])
                            nc.vector.scalar_tensor_tensor(
                                covc[:], rsb[:], rs_own[:], covc[:],
                                A.mult, A.add)
                    if ch == 15:
                        with tc.tile_wait_until(0.30):
                            t1 = hb_w.tile([128, 1024], F32, name="t1",
                                           tag="dt", bufs=2)
                            nc.vector.tensor_mul(t1[:], covc[:], rdb[:])
                            nc.vector.tensor_single_scalar(t1[:], t1[:],
                                                           rd_own[:], A.mult)
                            t1c = hb_w.tile([128, 1024], F32, name="t1c",
                                            tag="dt", bufs=2)
                            nc.scalar.activation(t1c[:], t1[:], AF.Abs,
                                                 accum_out=f1own[:])
                            nc.vector.tensor_single_scalar(f1own[:], f1own[:],
                                                           1.0 / float(B),
                                                           A.mult)

            # ---------------- Phase C: entropies + mask ----------------------
            with (
                tc.tile_pool(name="pc_w", bufs=2) as pc_w,
                tc.tile_pool(name="pc_ps", bufs=2, space="PSUM") as pc_ps,
                tc.tile_pool(name="pc_io", bufs=6) as pc_io,
            ):
                # prefetch mask-phase loads (independent of pvec)
                xm_t = []
                um_t = []
                CH = 2048
                for c in range(D // CH):
                    xm = pc_io.tile([128, CH], F32, name="xm", tag="xm", bufs=3)
                    um = pc_io.tile([128, CH], F32, name="um", tag="um", bufs=3)
                    nc.sync.dma_start(xm[:], xr.ap()[:, c * CH:(c + 1) * CH])
                    nc.sync.dma_start(um[:], ur.ap()[:, c * CH:(c + 1) * CH])
                    xm_t.append(xm)
                    um_t.append(um)

                # batch-count partial + exchange
                bc_part = pc_w.tile([32, 32], F32, name="bc_part", bufs=1)
                nc.vector.reduce_sum(
                    bc_part[:],
                    counts_sb[:].rearrange("p (r l) -> p l r", r=RSL), axis=AX)
                nc.sync.dma_start(bc_in[:], bc_part[:])
                nc.gpsimd.collective_compute(
                    "AllGather", A.bypass,
                    replica_groups=[list(range(N_CORES))],
                    ins=[bc_in.opt()], outs=[bc_out.opt()])

                # local entropies from accumulated partials (hidden under AG)
                psS = pc_ps.tile([1, RSL], F32, name="psS")
                nc.tensor.matmul(psS[:], ones32[:], erp_all[:], start=True,
                                 stop=True)
                srow = pc_w.tile([1, RSL], F32, name="srow", bufs=1)
                nc.scalar.copy(srow[:], psS[:])
                psT = pc_ps.tile([128, 1], F32, name="psT")
                nc.tensor.transpose(psT[:], srow[:], id128[:1, :1])
                hloc = pc_w.tile([128, 1], F32, name="hloc", bufs=1)
                nc.scalar.copy(hloc[:], psT[:])
                nc.vector.tensor_mul(hloc[:], hloc[:], scal[:, 14:15])

                agb = pc_w.tile([32, 8, 32], F32, name="agb", bufs=1)
                nc.sync.dma_start(
                    agb[:], bc_out[:].rearrange("(r p) c -> p r c", p=32))
                bcs = pc_w.tile([32, 32], F32, name="bcs", bufs=1)
                nc.vector.reduce_sum(
                    bcs[:], agb[:].rearrange("p r c -> p c r"), axis=AX)
                lnb = pc_w.tile([32, 32], F32, name="lnb", bufs=1)
                nc.scalar.activation(lnb[:], bcs[:], AF.Ln,
                                     scale=scal[0:32, 13:14], bias=epsb[0:32, :])
                nc.vector.tensor_mul(lnb[:], lnb[:], bcs[:])
                sb1 = pc_w.tile([32, 1], F32, name="sb1", bufs=1)
                nc.vector.reduce_sum(sb1[:], lnb[:], axis=AX)
                nc.gpsimd.partition_all_reduce(sb1[:], sb1[:], 32, _rop("add"))
                hbat = pc_w.tile([128, 1], F32, name="hbat", bufs=1)
                nc.gpsimd.partition_broadcast(hbat[:], sb1[0:1, :])
                nc.vector.tensor_mul(hbat[:], hbat[:], scal[:, 15:16])

                # f2' = max(f2, 1/f2); keep = f1/f2'; p = 1-keep; rkeep = 1/keep
                tA = pc_w.tile([128, 1], F32, name="tA", bufs=1)
                tB = pc_w.tile([128, 1], F32, name="tB", bufs=1)
                nc.vector.reciprocal(tA[:], hbat[:])
                f2 = pc_w.tile([128, 1], F32, name="f2", bufs=1)
                nc.vector.tensor_mul(f2[:], hloc[:], tA[:])
                nc.vector.reciprocal(tB[:], f2[:])
                nc.vector.tensor_max(f2[:], f2[:], tB[:])
                nc.vector.reciprocal(tB[:], f2[:])
                nc.vector.tensor_mul(pvec[:, 2:3], f1own[:], tB[:])
                nc.vector.tensor_scalar(pvec[:, 0:1], pvec[:, 2:3], -1.0, 1.0,
                                        A.mult, A.add)
                nc.vector.reciprocal(pvec[:, 1:2], pvec[:, 2:3])

                if DEBUG:
                    dbg_sb = pc_w.tile([128, 16], F32, name="dbg_sb", bufs=1)
                    nc.scalar.copy(dbg_sb[:, 0:4], pvec[:])
                    nc.scalar.copy(dbg_sb[:, 4:5], hloc[:])
                    nc.scalar.copy(dbg_sb[:, 5:6], hbat[:])
                    nc.scalar.copy(dbg_sb[:, 6:14], scal[:, 4:12])
                    nc.scalar.copy(dbg_sb[:, 14:15], f1own[:])
                    nc.scalar.copy(dbg_sb[:, 15:16], rd_own[:])
                    nc.sync.dma_start(dbg.ap(), dbg_sb[:])

                # mask + scale
                for c in range(D // CH):
                    xm, um = xm_t[c], um_t[c]
                    nc.vector.tensor_single_scalar(um[:], um[:], pvec[:, 0:1],
                                                   A.is_gt)
                    oc = pc_io.tile([128, CH], F32, name="oc", tag="oc", bufs=3)
                    nc.vector.scalar_tensor_tensor(oc[:], um[:], pvec[:, 1:2],
                                                   xm[:], A.mult, A.mult)
                    nc.sync.dma_start(out.ap()[:, c * CH:(c + 1) * CH], oc[:])

    nc.compile()
    return nc


def _rop(name):
    from concourse import bass_isa
    return getattr(bass_isa.ReduceOp, name)


def _bcast_cols(nc, sbuf_pool, psum_pool, vec8, dst, id128):
    """dst[p, t*128+q] = vec8[q, t]  (flatten [128,8] col-major, bcast to all
    partitions)."""
    import concourse.mybir as mybir
    F32 = mybir.dt.float32
    pt = psum_pool.tile([8, 128], F32, name="bc_pt")
    nc.tensor.transpose(pt[:8, :], vec8[:], id128[:])
    tr = sbuf_pool.tile([8, 128], F32, name="bc_tr", bufs=1)
    nc.scalar.copy(tr[:], pt[:8, :])
    flat = sbuf_pool.tile([1, 8 * 128], F32, name="bc_flat", bufs=1)
    for t in range(8):
        nc.sync.dma_start(flat[:, t * 128:(t + 1) * 128], tr[t:t + 1, :])
    nc.gpsimd.partition_broadcast(dst[:], flat[:])


def kernel(x, u):
    if "nc" not in _cache:
        _cache["nc"] = _build()
    nc = _cache["nc"]
    from concourse.bass_utils import run_bass_kernel_spmd

    x = np.asarray(x, dtype=np.float32)
    u = np.asarray(u, dtype=np.float32)
    orig_shape = x.shape
    xf = np.ascontiguousarray(x.reshape(B, D))
    uf = np.ascontiguousarray(u.reshape(B, D))
    in_maps = []
    for c in range(N_CORES):
        selv = np.zeros((128, 8), np.float32)
        selv[:, c] = 1.0
        in_maps.append({
            "xst": np.ascontiguousarray(xf[:, c * DSL:(c + 1) * DSL].T),
            "xr": np.ascontiguousarray(xf[c * RSL:(c + 1) * RSL, :]),
            "ur": np.ascontiguousarray(uf[c * RSL:(c + 1) * RSL, :]),
            "sel": selv,
        })
    res = run_bass_kernel_spmd(nc, in_maps, core_ids=list(range(N_CORES)))
    _cache["last_results"] = res
    outf = np.concatenate([res.results[c]["out"] for c in range(N_CORES)], axis=0)
    return outf.reshape(orig_shape)
